# revision 1
# baseline (speedup 1.0000x reference)
"""Trainium2 Bass kernel for nn_MultiHeadAttention (B=8192, D=1024, 16 heads
used only via the softmax scale 1/8).

Strategy (8 NeuronCores, zero inter-core communication):
  - Rows (batch axis) of the attention output are sharded: core c owns rows
    [c*1024, (c+1)*1024).
  - Every core recomputes the full K^T and V projections for all 8192 rows
    (replicated compute instead of an all-gather; collectives on this part
    are slower than the 2x17 GFLOP of extra matmul).
  - Attention runs in a transposed-energy ("E^T") layout so no probability
    transpose is ever needed:
        E^T[j, i] = sum_o K^T[o, j] * Q^T[o, i]
        P^T = exp(E^T * 0.125)            (no max subtraction; |E|<40, safe)
        out_unnorm[i, o] = sum_j P^T[j, i] * V[j, o]
        s[i] = sum_j P^T[j, i]            (matmul against a ones vector)
        out = out_unnorm / s + bv         (bv folded in post-normalization)
  - All big matmuls run in float32r (full-rate streaming on the PE at
    N=512) with fp32 PSUM accumulation.
"""

import sys

sys.path.insert(0, "/opt/trn_rl_repo")

import numpy as np

import concourse.bass as bass  # noqa: F401
import concourse.tile as tile
from concourse import bacc, mybir
from concourse.bass_utils import run_bass_kernel_spmd
from concourse.masks import make_identity

B = 8192
D = 1024
P = 128
NCORES = 8
R = B // NCORES  # 1024 rows per core
JBLK = 512  # j-block (keys/values) streamed per iteration
NJB = B // JBLK  # 16
DO = D // P  # 8 feature chunks of 128
IC = R // P  # 8 row chunks of 128 per core
F32 = mybir.dt.float32
F32R = mybir.dt.float32r
BF16 = mybir.dt.bfloat16
AF = mybir.ActivationFunctionType
ALU = mybir.AluOpType
SCALE = 0.125  # 1/sqrt(head_dim=64)




def _transpose_rows_to_sbuf(nc, tp_psum, identity, row_sb, xt_dst, jj):
    """PE-transpose a [128, 1024] natural-layout row chunk into the
    [128(d_in), DO, ...] transposed SBUF tile at free offset jj*128."""
    for dd in range(DO):
        tp = tp_psum.tile([P, P], F32, tag="tp")
        nc.tensor.transpose(tp, row_sb[:, dd * P : (dd + 1) * P], identity)
        nc.vector.tensor_copy(
            out=xt_dst[:, dd, jj * P : (jj + 1) * P], in_=tp
        )


def build_program():
    nc = bacc.Bacc(
        "TRN2", target_bir_lowering=False, debug=False, num_devices=NCORES
    )
    x = nc.dram_tensor("x", [B, D], F32, kind="ExternalInput").ap()
    x_loc = nc.dram_tensor("x_loc", [R, D], F32, kind="ExternalInput").ap()
    w_q = nc.dram_tensor("Wq", [D, D], F32, kind="ExternalInput").ap()
    w_k = nc.dram_tensor("Wk", [D, D], F32, kind="ExternalInput").ap()
    w_v = nc.dram_tensor("Wv", [D, D], F32, kind="ExternalInput").ap()
    b_q = nc.dram_tensor("bq", [D], F32, kind="ExternalInput").ap()
    b_k = nc.dram_tensor("bk", [D], F32, kind="ExternalInput").ap()
    b_v = nc.dram_tensor("bv", [D], F32, kind="ExternalInput").ap()
    out_loc = nc.dram_tensor("out_loc", [R, D], F32, kind="ExternalOutput").ap()

    with tile.TileContext(nc) as tc:
        _body(nc, tc, x, x_loc, w_q, w_k, w_v, b_q, b_k, b_v, out_loc)
    nc.compile()
    return nc


def _body(nc, tc, x, x_loc, w_q, w_k, w_v, b_q, b_k, b_v, out_loc):
    from contextlib import ExitStack

    outer = ExitStack()
    outer.__enter__()
    # ---- persistent pools (whole kernel) ----
    const_pool = outer.enter_context(tc.tile_pool(name="const", bufs=1))
    identity = const_pool.tile([P, P], F32)
    make_identity(nc, identity)
    ones_f32 = const_pool.tile([P, 2], F32)
    nc.vector.memset(ones_f32, 1.0)
    ones = const_pool.tile([P, 2], BF16)
    nc.vector.tensor_copy(out=ones, in_=ones_f32)
    bq_sb = const_pool.tile([P, DO], F32)
    nc.sync.dma_start(bq_sb, b_q.rearrange("(oo p) -> p oo", p=P))
    bk_sb = const_pool.tile([P, DO], F32)
    nc.sync.dma_start(bk_sb, b_k.rearrange("(oo p) -> p oo", p=P))
    ones_row = const_pool.tile([1, P], F32)
    nc.vector.memset(ones_row, 1.0)
    # broadcast bv across all 128 partitions with a K=1 matmul:
    # load bv into partition 0 of bv_bc, then out[p, o] = 1 * bv[o]
    bv_bc = const_pool.tile([P, D], F32)
    nc.sync.dma_start(bv_bc[0:1, :], b_v[None, :])
    with tc.tile_pool(name="bv_psum", bufs=2, space="PSUM") as bvp:
        for oh in range(2):
            pt = bvp.tile([P, 512], F32, tag="bvp")
            nc.tensor.matmul(
                pt,
                ones_row,
                bv_bc[0:1, oh * 512 : (oh + 1) * 512],
                start=True,
                stop=True,
            )
            nc.vector.tensor_copy(out=bv_bc[:, oh * 512 : (oh + 1) * 512], in_=pt)

    qt_pool = outer.enter_context(tc.tile_pool(name="qt", bufs=1))
    qt = qt_pool.tile([P, DO, R], F32R)  # Q^T: [o_in, o_out, i]  (4 MB)

    sums_pool = outer.enter_context(tc.tile_pool(name="sums", bufs=1))
    sums_acc = sums_pool.tile([P, 2 * IC], F32)  # per-row exp-sums (even cols)
    rsum = sums_pool.tile([P, 2 * IC], F32)

    # DRAM scratch for the full K^T / V (32 MB each) — streamed in phase 2.
    dram = outer.enter_context(tc.tile_pool(name="dram", bufs=1, space="DRAM"))
    kt_dram = dram.tile([DO, P, B], F32R)  # K^T: [o_out][o_in][j]
    v_dram = dram.tile([B, D], BF16)  # V: natural [j, o]

    # =========================================================
    # Phase 0+1: weight transposes, Q^T (local), K^T/V (full)
    # =========================================================
    with ExitStack() as p1:
        wt_pool = p1.enter_context(tc.tile_pool(name="wt", bufs=1))
        wqt = wt_pool.tile([P, DO, D], F32R)  # W^T: [d_in, d_out, o] (4 MB)
        wkt = wt_pool.tile([P, DO, D], F32R)
        wvt = wt_pool.tile([P, DO, D], BF16)

        row_pool = p1.enter_context(tc.tile_pool(name="rows", bufs=2))
        xt_pool = p1.enter_context(tc.tile_pool(name="xt", bufs=2))
        st_pool = p1.enter_context(tc.tile_pool(name="stage", bufs=2))
        tp_psum = p1.enter_context(tc.tile_pool(name="tp_ps", bufs=2, space="PSUM"))
        mm_psum = p1.enter_context(tc.tile_pool(name="mm_ps", bufs=4, space="PSUM"))

        # -- transpose the three weight matrices into SBUF --
        for wt_sb, w_dram in ((wqt, w_q), (wkt, w_k), (wvt, w_v)):
            for oo in range(DO):
                wrow = row_pool.tile([P, D], F32, tag="row")
                nc.sync.dma_start(wrow, w_dram[oo * P : (oo + 1) * P, :])
                for dd in range(DO):
                    tp = tp_psum.tile([P, P], F32, tag="tp")
                    nc.tensor.transpose(
                        tp, wrow[:, dd * P : (dd + 1) * P], identity
                    )
                    nc.vector.tensor_copy(
                        out=wt_sb[:, dd, oo * P : (oo + 1) * P], in_=tp
                    )

        # -- Q^T for the local shard, in halves of 512 rows --
        for ih in range(R // JBLK):
            xt_blk = xt_pool.tile([P, DO, JBLK], F32R, tag="xt")
            for jj in range(JBLK // P):
                xrow = row_pool.tile([P, D], F32, tag="row")
                nc.sync.dma_start(
                    xrow, x_loc[(ih * 4 + jj) * P : (ih * 4 + jj + 1) * P, :]
                )
                _transpose_rows_to_sbuf(nc, tp_psum, identity, xrow, xt_blk, jj)
            for oo in range(DO):
                pq = mm_psum.tile([P, JBLK], F32, tag="mm")
                for dd in range(DO):
                    nc.tensor.matmul(
                        pq,
                        (wqt[:, dd, oo * P : (oo + 1) * P]),
                        (xt_blk[:, dd, :]),
                        start=(dd == 0),
                        stop=(dd == DO - 1),
                    )
                nc.scalar.activation(
                    qt[:, oo, ih * JBLK : (ih + 1) * JBLK],
                    pq,
                    AF.Identity,
                    bias=bq_sb[:, oo : oo + 1],
                )

        # -- full K^T and V, streamed over 16 j-blocks of 512 rows --
        for jb in range(NJB):
            xt_blk = xt_pool.tile([P, DO, JBLK], F32R, tag="xt")
            for jj in range(JBLK // P):
                xrow = row_pool.tile([P, D], F32, tag="row")
                nc.sync.dma_start(
                    xrow, x[(jb * 4 + jj) * P : (jb * 4 + jj + 1) * P, :]
                )
                _transpose_rows_to_sbuf(nc, tp_psum, identity, xrow, xt_blk, jj)
            xt_bf = xt_pool.tile([P, DO, JBLK], BF16, tag="xtb")
            nc.vector.tensor_copy(out=xt_bf, in_=xt_blk)
            # K^T block: [o, j]
            for oo in range(DO):
                pk = mm_psum.tile([P, JBLK], F32, tag="mm")
                for dd in range(DO):
                    nc.tensor.matmul(
                        pk,
                        (wkt[:, dd, oo * P : (oo + 1) * P]),
                        (xt_blk[:, dd, :]),
                        start=(dd == 0),
                        stop=(dd == DO - 1),
                    )
                kst = st_pool.tile([P, JBLK], F32R, tag="kst")
                nc.scalar.activation(
                    kst, pk, AF.Identity, bias=bk_sb[:, oo : oo + 1]
                )
                nc.sync.dma_start(
                    kt_dram[oo, :, jb * JBLK : (jb + 1) * JBLK], kst
                )
            # V block: natural [j, o], bias deferred to the epilogue
            for jj in range(JBLK // P):
                vst = st_pool.tile([P, D], BF16, tag="vst")
                pv_h = [mm_psum.tile([P, JBLK], F32, tag="mm", name="pv") for _ in range(2)]
                for dd in range(DO):
                    for oh in range(2):
                        nc.tensor.matmul(
                            pv_h[oh],
                            (xt_bf[:, dd, jj * P : (jj + 1) * P]),
                            (wvt[:, dd, oh * 512 : (oh + 1) * 512]),
                            start=(dd == 0),
                            stop=(dd == DO - 1),
                        )
                for oh in range(2):
                    nc.vector.tensor_copy(
                        out=vst[:, oh * 512 : (oh + 1) * 512], in_=pv_h[oh]
                    )
                nc.sync.dma_start(
                    v_dram[jb * JBLK + jj * P : jb * JBLK + (jj + 1) * P, :], vst
                )

    # =========================================================
    # Phase 2: streamed attention in E^T layout
    # =========================================================
    with ExitStack() as p2:
        oa_pool = p2.enter_context(tc.tile_pool(name="oacc", bufs=1))
        outacc = oa_pool.tile([P, IC, D], F32)  # 4 MB

        kt_pool = p2.enter_context(tc.tile_pool(name="ktb", bufs=3))
        v_pool = p2.enter_context(tc.tile_pool(name="vtb", bufs=3))
        pt_pool = p2.enter_context(tc.tile_pool(name="ptb", bufs=3))
        e_psum = p2.enter_context(tc.tile_pool(name="e_ps", bufs=4, space="PSUM"))
        o_psum = p2.enter_context(tc.tile_pool(name="o_ps", bufs=3, space="PSUM"))
        s_psum = p2.enter_context(tc.tile_pool(name="s_ps", bufs=1, space="PSUM"))

        for jb in range(NJB):
            ktb = kt_pool.tile([P, DO, JBLK], F32R, tag="ktb")
            for oo in range(DO):
                nc.sync.dma_start(
                    ktb[:, oo, :], kt_dram[oo, :, jb * JBLK : (jb + 1) * JBLK]
                )
            vtb = v_pool.tile([P, JBLK // P, D], BF16, tag="vtb")
            nc.sync.dma_start(
                vtb,
                v_dram[jb * JBLK : (jb + 1) * JBLK, :].rearrange(
                    "(jj p) o -> p jj o", p=P
                ),
            )
            # unnormalized probabilities P^T for this j-block: [j, i]
            ptb = pt_pool.tile([P, JBLK // P, R], BF16, tag="ptb")
            for jj in range(JBLK // P):
                pe_h = [
                    e_psum.tile([P, JBLK], F32, tag="pe", name="pe")
                    for _ in range(R // JBLK)
                ]
                for oo in range(DO):
                    for ih in range(R // JBLK):
                        nc.tensor.matmul(
                            pe_h[ih],
                            (ktb[:, oo, jj * P : (jj + 1) * P]),
                            (qt[:, oo, ih * JBLK : (ih + 1) * JBLK]),
                            start=(oo == 0),
                            stop=(oo == DO - 1),
                        )
                for ih in range(R // JBLK):
                    nc.scalar.activation(
                        ptb[:, jj, ih * JBLK : (ih + 1) * JBLK],
                        pe_h[ih],
                        AF.Exp,
                        scale=SCALE,
                    )
            # row sums of P^T (reduce over j): matmul against ones
            # out_unnorm += P^T.T @ V, with the exp-sums matmul sharing each
            # stationary ptb tile (3 streams per weight load)
            ps = s_psum.tile([P, 2 * IC], F32, tag="ps")
            for ic in range(IC):
                po_h = [o_psum.tile([P, 512], F32, tag="po", name="po") for _ in range(2)]
                for jj in range(JBLK // P):
                    for oh in range(2):
                        nc.tensor.matmul(
                            po_h[oh],
                            (ptb[:, jj, ic * P : (ic + 1) * P]),
                            (vtb[:, jj, oh * 512 : (oh + 1) * 512]),
                            start=(jj == 0),
                            stop=(jj == JBLK // P - 1),
                        )
                    nc.tensor.matmul(
                        ps[:, 2 * ic : 2 * ic + 2],
                        (ptb[:, jj, ic * P : (ic + 1) * P]),
                        (ones),
                        start=(ic == 0 and jj == 0),
                        stop=(ic == IC - 1 and jj == JBLK // P - 1),
                    )
                for oh in range(2):
                    dst = outacc[:, ic, oh * 512 : (oh + 1) * 512]
                    if jb == 0:
                        nc.vector.tensor_copy(out=dst, in_=po_h[oh])
                    else:
                        nc.vector.tensor_tensor(dst, po_h[oh], dst, ALU.add)
            if jb == 0:
                nc.vector.tensor_copy(out=sums_acc, in_=ps)
            else:
                nc.vector.tensor_tensor(sums_acc, ps, sums_acc, ALU.add)

        # ---- epilogue: normalize, add bv, write out ----
        nc.vector.reciprocal(rsum, sums_acc)
        fin_pool = p2.enter_context(tc.tile_pool(name="fin", bufs=2))
        for ic in range(IC):
            ofin = fin_pool.tile([P, D], F32, tag="ofin")
            nc.vector.tensor_scalar_mul(ofin, outacc[:, ic, :], rsum[:, 2 * ic : 2 * ic + 1])
            nc.vector.tensor_tensor(ofin, ofin, bv_bc, ALU.add)
            nc.sync.dma_start(out_loc[ic * P : (ic + 1) * P, :], ofin)

    outer.close()


_NC_CACHE = None


def _get_program():
    global _NC_CACHE
    if _NC_CACHE is None:
        _NC_CACHE = build_program()
    return _NC_CACHE


def _run(inputs, trace=False):
    nc = _get_program()
    x = np.ascontiguousarray(np.asarray(inputs["x"], dtype=np.float32))
    common = {
        k: np.ascontiguousarray(np.asarray(inputs[k], dtype=np.float32))
        for k in ("Wq", "Wk", "Wv", "bq", "bk", "bv")
    }
    in_maps = [
        {"x": x, "x_loc": np.ascontiguousarray(x[c * R : (c + 1) * R]), **common}
        for c in range(NCORES)
    ]
    res = run_bass_kernel_spmd(
        nc, in_maps, core_ids=list(range(NCORES)), trace=trace
    )
    out = np.concatenate([res.results[c]["out_loc"] for c in range(NCORES)], axis=0)
    return out.reshape(B, D, 1).astype(np.float32), res


def kernel(**inputs):
    out, _ = _run(inputs, trace=False)
    return out



# revision 2
# speedup vs baseline: 1.0341x; 1.0341x over previous
"""Trainium2 Bass kernel for nn_MultiHeadAttention (B=8192, D=1024, 16 heads
used only via the softmax scale 1/8).

Strategy (8 NeuronCores, AllGather of locally-projected K^T only):
  - Rows (batch axis) are sharded: core c owns rows [c*1024, (c+1)*1024).
  - Each core computes Q^T and K^T for ONLY its local 1024 rows (fp16
    weights/activations, fp32 PSUM), then AllGathers K^T (fp16, two halves
    so the first half's attention can start while the second is on the
    wire). Collectives run on TOPSP/SDMA, overlapped with Wq/Wv transposes
    and the Q^T projection.
  - V is NEVER materialized: since V = x @ Wv.T + bv,
        attn @ V = (attn @ x) @ Wv.T + (rowsum attn) * bv
    so the j-contraction runs against the raw input x (full x is a kernel
    input on every core - no gather needed), and the Wv projection is
    applied once to the [1024, 1024] attn@x result in the epilogue.
  - Attention in transposed-energy ("E^T") layout, no probability
    transpose needed:
        E^T[j, i] = sum_o K^T[o, j] * Q^T[o, i]      (fp16 x fp16)
        P^T = exp(E^T * 0.125)        bf16 (range!); |E|<40 pre-exp safe
        AX[i, d] = sum_j P^T[j, i] * x[j, d]         (bf16 x bf16)
        s[i] = sum_j P^T[j, i]        (matmul against a ones vector,
                                       issued FIRST in each group so the
                                       next ldweights hides under the two
                                       512-row matmuls)
        out = (AX / s) @ Wv.T + bv                   (fp16 x fp16)
  - Phase-2 stream pools are hoisted to the outer scope so their SBUF is
    never re-used from phase-1 pools: the K/x prefetch DMAs start during
    phase 1 instead of waiting for its drain.
"""

import sys

sys.path.insert(0, "/opt/trn_rl_repo")

import numpy as np

import concourse.bass as bass  # noqa: F401
import concourse.tile as tile
from concourse import bacc, mybir
from concourse.bass_utils import run_bass_kernel_spmd
from concourse.masks import make_identity

B = 8192
D = 1024
P = 128
NCORES = 8
R = B // NCORES  # 1024 rows per core
JBLK = 512  # j-block (keys) streamed per iteration
NJB = B // JBLK  # 16
DO = D // P  # 8 feature chunks of 128
IC = R // P  # 8 row chunks of 128 per core
NH = 2  # K^T allgather halves
F32 = mybir.dt.float32
FP16 = mybir.dt.float16
BF16 = mybir.dt.bfloat16
AF = mybir.ActivationFunctionType
ALU = mybir.AluOpType
SCALE = 0.125  # 1/sqrt(head_dim=64)
RG = [list(range(NCORES))]


def build_program():
    nc = bacc.Bacc(
        "TRN2", target_bir_lowering=False, debug=False, num_devices=NCORES
    )
    x = nc.dram_tensor("x", [B, D], F32, kind="ExternalInput").ap()
    x_loc = nc.dram_tensor("x_loc", [R, D], F32, kind="ExternalInput").ap()
    w_q = nc.dram_tensor("Wq", [D, D], F32, kind="ExternalInput").ap()
    w_k = nc.dram_tensor("Wk", [D, D], F32, kind="ExternalInput").ap()
    w_v = nc.dram_tensor("Wv", [D, D], F32, kind="ExternalInput").ap()
    b_q = nc.dram_tensor("bq", [D], F32, kind="ExternalInput").ap()
    b_k = nc.dram_tensor("bk", [D], F32, kind="ExternalInput").ap()
    b_v = nc.dram_tensor("bv", [D], F32, kind="ExternalInput").ap()
    out_loc = nc.dram_tensor("out_loc", [R, D], F32, kind="ExternalOutput").ap()

    with tile.TileContext(nc) as tc:
        _body(nc, tc, x, x_loc, w_q, w_k, w_v, b_q, b_k, b_v, out_loc)
    nc.compile()
    return nc


def _body(nc, tc, x, x_loc, w_q, w_k, w_v, b_q, b_k, b_v, out_loc):
    from contextlib import ExitStack

    outer = ExitStack()
    outer.__enter__()
    # ---- persistent pools (whole kernel) ----
    const_pool = outer.enter_context(tc.tile_pool(name="const", bufs=1))
    identity = const_pool.tile([P, P], F32)
    make_identity(nc, identity)
    ones_tmp = const_pool.tile([P, 2], F32)
    nc.vector.memset(ones_tmp, 1.0)
    ones_bf = const_pool.tile([P, 2], BF16)
    nc.vector.tensor_copy(out=ones_bf, in_=ones_tmp)
    bq_sb = const_pool.tile([P, DO], F32)
    nc.sync.dma_start(bq_sb, b_q.rearrange("(oo p) -> p oo", p=P))
    bk_sb = const_pool.tile([P, DO], F32)
    nc.sync.dma_start(bk_sb, b_k.rearrange("(oo p) -> p oo", p=P))
    ones_row = const_pool.tile([1, P], F32)
    nc.vector.memset(ones_row, 1.0)
    # broadcast bv across all 128 partitions with a K=1 matmul:
    # load bv into partition 0 of bv_bc, then out[p, o] = 1 * bv[o]
    bv_bc = const_pool.tile([P, D], F32)
    nc.sync.dma_start(bv_bc[0:1, :], b_v[None, :])
    with tc.tile_pool(name="bv_psum", bufs=2, space="PSUM") as bvp:
        for oh in range(2):
            pt = bvp.tile([P, 512], F32, tag="bvp")
            nc.tensor.matmul(
                pt,
                ones_row,
                bv_bc[0:1, oh * 512 : (oh + 1) * 512],
                start=True,
                stop=True,
            )
            nc.vector.tensor_copy(out=bv_bc[:, oh * 512 : (oh + 1) * 512], in_=pt)

    qt_pool = outer.enter_context(tc.tile_pool(name="qt", bufs=1))
    qt = qt_pool.tile([P, DO, R], FP16)  # Q^T: [o_in, o_out, i]  (2 MB)
    wvt_pool = outer.enter_context(tc.tile_pool(name="wvt", bufs=1))
    wvt = wvt_pool.tile([P, DO, D], FP16)  # Wv^T, used only in the epilogue

    sums_pool = outer.enter_context(tc.tile_pool(name="sums", bufs=1))
    sums_acc = sums_pool.tile([P, 2 * IC], F32)  # per-row exp-sums (even cols)
    rsum = sums_pool.tile([P, 2 * IC], F32)

    # phase-2 stream pools, hoisted so their SBUF never aliases phase-1
    # pools (prefetch DMAs may start while phase 1 is still draining)
    oa_pool = outer.enter_context(tc.tile_pool(name="oacc", bufs=1))
    outacc = oa_pool.tile([P, IC, D], F32)  # AX accumulator (4 MB)
    kt_pool = outer.enter_context(tc.tile_pool(name="ktb", bufs=2))
    xr_pool = outer.enter_context(tc.tile_pool(name="xraw", bufs=2))
    xb_pool = outer.enter_context(tc.tile_pool(name="xb", bufs=2))
    pt_pool = outer.enter_context(tc.tile_pool(name="ptb", bufs=1))

    # DRAM: local K^T shard (two halves) + gathered full copies.
    dram = outer.enter_context(tc.tile_pool(name="dram", bufs=1, space="DRAM"))
    kt_part = [dram.tile([DO, P, JBLK], FP16, name=f"kt_part{h}") for h in range(NH)]
    kt_full = [
        dram.tile([NCORES, DO, P, JBLK], FP16, name=f"kt_full{h}") for h in range(NH)
    ]

    # =========================================================
    # Phase 1: weight transposes, local K^T / Q^T, AllGathers
    # =========================================================
    with ExitStack() as p1:
        wt_pool = p1.enter_context(tc.tile_pool(name="wt", bufs=1))
        xt_pool = p1.enter_context(tc.tile_pool(name="xt", bufs=1))
        xt = xt_pool.tile([P, DO, R], FP16)  # x_loc^T (2 MB)

        row_pool = p1.enter_context(tc.tile_pool(name="rows", bufs=2))
        st_pool = p1.enter_context(tc.tile_pool(name="stage", bufs=2))
        tp_psum = p1.enter_context(tc.tile_pool(name="tp_ps", bufs=2, space="PSUM"))
        mm_psum = p1.enter_context(tc.tile_pool(name="mm_ps", bufs=4, space="PSUM"))

        def transpose_weight(wt_sb, w_dram):
            for oo in range(DO):
                wrow = row_pool.tile([P, D], F32, tag="row")
                nc.sync.dma_start(wrow, w_dram[oo * P : (oo + 1) * P, :])
                for dd in range(DO):
                    tp = tp_psum.tile([P, P], F32, tag="tp")
                    nc.tensor.transpose(
                        tp, wrow[:, dd * P : (dd + 1) * P], identity
                    )
                    nc.vector.tensor_copy(
                        out=wt_sb[:, dd, oo * P : (oo + 1) * P], in_=tp
                    )

        # -- Wk first: it gates the collectives --
        wkt = wt_pool.tile([P, DO, D], FP16, tag="wt", name="wkt")
        transpose_weight(wkt, w_k)

        # -- x_loc^T for the local shard --
        for jj in range(R // P):
            xrow = row_pool.tile([P, D], F32, tag="row")
            nc.sync.dma_start(xrow, x_loc[jj * P : (jj + 1) * P, :])
            for dd in range(DO):
                tp = tp_psum.tile([P, P], F32, tag="tp")
                nc.tensor.transpose(tp, xrow[:, dd * P : (dd + 1) * P], identity)
                nc.vector.tensor_copy(out=xt[:, dd, jj * P : (jj + 1) * P], in_=tp)

        # -- local K^T shard halves: [o, j_loc], AllGather each asap --
        for h in range(NH):
            for oo in range(DO):
                pk = mm_psum.tile([P, JBLK], F32, tag="mm")
                for dd in range(DO):
                    nc.tensor.matmul(
                        pk,
                        (wkt[:, dd, oo * P : (oo + 1) * P]),
                        (xt[:, dd, h * JBLK : (h + 1) * JBLK]),
                        start=(dd == 0),
                        stop=(dd == DO - 1),
                    )
                kst = st_pool.tile([P, JBLK], FP16, tag="kst")
                nc.scalar.activation(
                    kst, pk, AF.Identity, bias=bk_sb[:, oo : oo + 1]
                )
                nc.sync.dma_start(kt_part[h][oo, :, :], kst)
            nc.gpsimd.collective_compute(
                "AllGather",
                mybir.AluOpType.bypass,
                replica_groups=RG,
                ins=[kt_part[h][:, :, :]],
                outs=[kt_full[h][:, :, :, :]],
            )

        # -- Q^T for the local shard (overlaps the in-flight collectives) --
        wqt = wt_pool.tile([P, DO, D], FP16, tag="wt", name="wqt")
        transpose_weight(wqt, w_q)
        for ih in range(R // JBLK):
            for oo in range(DO):
                pq = mm_psum.tile([P, JBLK], F32, tag="mm")
                for dd in range(DO):
                    nc.tensor.matmul(
                        pq,
                        (wqt[:, dd, oo * P : (oo + 1) * P]),
                        (xt[:, dd, ih * JBLK : (ih + 1) * JBLK]),
                        start=(dd == 0),
                        stop=(dd == DO - 1),
                    )
                nc.scalar.activation(
                    qt[:, oo, ih * JBLK : (ih + 1) * JBLK],
                    pq,
                    AF.Identity,
                    bias=bq_sb[:, oo : oo + 1],
                )

        # -- Wv^T (only needed by the epilogue; do it last) --
        transpose_weight(wvt, w_v)

    # =========================================================
    # Phase 2: streamed attention in E^T layout, j against raw x
    # =========================================================
    with ExitStack() as p2:
        p2m = ExitStack()
        p2m.__enter__()
        e_psum = p2m.enter_context(tc.tile_pool(name="e_ps", bufs=4, space="PSUM"))
        o_psum = p2m.enter_context(tc.tile_pool(name="o_ps", bufs=3, space="PSUM"))
        s_psum = p2m.enter_context(tc.tile_pool(name="s_ps", bufs=1, space="PSUM"))

        for it in range(NJB):
            # half-major order: all of AG half 0 first, then half 1
            src_h, src_c = it // NCORES, it % NCORES
            jb = src_c * 2 + src_h
            ktb = kt_pool.tile([P, DO, JBLK], FP16, tag="ktb")
            for oo in range(DO):
                nc.sync.dma_start(
                    ktb[:, oo, :], kt_full[src_h][src_c, oo, :, :]
                )
            xraw = xr_pool.tile([P, JBLK // P, D], F32, tag="xraw")
            nc.sync.dma_start(
                xraw,
                x[jb * JBLK : (jb + 1) * JBLK, :].rearrange(
                    "(jj p) o -> p jj o", p=P
                ),
            )
            xb = xb_pool.tile([P, JBLK // P, D], BF16, tag="xb")
            nc.gpsimd.tensor_copy(out=xb, in_=xraw)  # Pool engine: cast
            # unnormalized probabilities P^T for this j-block: [j, i]
            ptb = pt_pool.tile([P, JBLK // P, R], BF16, tag="ptb")
            for jj in range(JBLK // P):
                pe_h = [
                    e_psum.tile([P, JBLK], F32, tag="pe", name="pe")
                    for _ in range(R // JBLK)
                ]
                for oo in range(DO):
                    for ih in range(R // JBLK):
                        nc.tensor.matmul(
                            pe_h[ih],
                            (ktb[:, oo, jj * P : (jj + 1) * P]),
                            (qt[:, oo, ih * JBLK : (ih + 1) * JBLK]),
                            start=(oo == 0),
                            stop=(oo == DO - 1),
                        )
                for ih in range(R // JBLK):
                    nc.scalar.activation(
                        ptb[:, jj, ih * JBLK : (ih + 1) * JBLK],
                        pe_h[ih],
                        AF.Exp,
                        scale=SCALE,
                    )
            # AX += P^T.T @ x_blk. The 2-row exp-sums matmul runs FIRST in
            # each stationary group so the NEXT group's ldweights hides
            # under this group's two 512-row matmuls.
            ps = s_psum.tile([P, 2 * IC], F32, tag="ps")
            for ic in range(IC):
                po_h = [
                    o_psum.tile([P, 512], F32, tag="po", name="po")
                    for _ in range(2)
                ]
                for jj in range(JBLK // P):
                    nc.tensor.matmul(
                        ps[:, 2 * ic : 2 * ic + 2],
                        (ptb[:, jj, ic * P : (ic + 1) * P]),
                        (ones_bf),
                        start=(ic == 0 and jj == 0),
                        stop=(ic == IC - 1 and jj == JBLK // P - 1),
                    )
                    for oh in range(2):
                        nc.tensor.matmul(
                            po_h[oh],
                            (ptb[:, jj, ic * P : (ic + 1) * P]),
                            (xb[:, jj, oh * 512 : (oh + 1) * 512]),
                            start=(jj == 0),
                            stop=(jj == JBLK // P - 1),
                        )
                for oh in range(2):
                    dst = outacc[:, ic, oh * 512 : (oh + 1) * 512]
                    if it == 0:
                        nc.vector.tensor_copy(out=dst, in_=po_h[oh])
                    else:
                        nc.vector.tensor_tensor(dst, po_h[oh], dst, ALU.add)
            if it == 0:
                nc.vector.tensor_copy(out=sums_acc, in_=ps)
            else:
                nc.vector.tensor_tensor(sums_acc, ps, sums_acc, ALU.add)

        p2m.close()  # release main-loop PSUM pools for the epilogue's
        # ---- epilogue: normalize, transpose AX, @ Wv^T, add bv ----
        nc.vector.reciprocal(rsum, sums_acc)
        axt_pool = p2.enter_context(tc.tile_pool(name="axt", bufs=1))
        axt = axt_pool.tile([P, DO, R], FP16)  # (AX/s)^T: [d_in, dd, i] (2 MB)
        fin_pool = p2.enter_context(tc.tile_pool(name="fin", bufs=2))
        tp2_psum = p2.enter_context(tc.tile_pool(name="tp2", bufs=2, space="PSUM"))
        f_psum = p2.enter_context(tc.tile_pool(name="f_ps", bufs=4, space="PSUM"))
        for ic in range(IC):
            nc.vector.tensor_scalar_mul(
                outacc[:, ic, :], outacc[:, ic, :], rsum[:, 2 * ic : 2 * ic + 1]
            )
            for dd in range(DO):
                tp = tp2_psum.tile([P, P], F32, tag="tp2")
                nc.tensor.transpose(
                    tp, outacc[:, ic, dd * P : (dd + 1) * P], identity
                )
                nc.vector.tensor_copy(
                    out=axt[:, dd, ic * P : (ic + 1) * P], in_=tp
                )
        for ic in range(IC):
            ofin = fin_pool.tile([P, D], F32, tag="ofin")
            for oh in range(2):
                pf = f_psum.tile([P, 512], F32, tag="f")
                for dd in range(DO):
                    nc.tensor.matmul(
                        pf,
                        (axt[:, dd, ic * P : (ic + 1) * P]),
                        (wvt[:, dd, oh * 512 : (oh + 1) * 512]),
                        start=(dd == 0),
                        stop=(dd == DO - 1),
                    )
                nc.vector.tensor_tensor(
                    ofin[:, oh * 512 : (oh + 1) * 512],
                    pf,
                    bv_bc[:, oh * 512 : (oh + 1) * 512],
                    ALU.add,
                )
            nc.sync.dma_start(out_loc[ic * P : (ic + 1) * P, :], ofin)

    outer.close()


_NC_CACHE = None


def _get_program():
    global _NC_CACHE
    if _NC_CACHE is None:
        _NC_CACHE = build_program()
    return _NC_CACHE


def _run(inputs, trace=False):
    nc = _get_program()
    x = np.ascontiguousarray(np.asarray(inputs["x"], dtype=np.float32))
    common = {
        k: np.ascontiguousarray(np.asarray(inputs[k], dtype=np.float32))
        for k in ("Wq", "Wk", "Wv", "bq", "bk", "bv")
    }
    in_maps = [
        {"x": x, "x_loc": np.ascontiguousarray(x[c * R : (c + 1) * R]), **common}
        for c in range(NCORES)
    ]
    res = run_bass_kernel_spmd(
        nc, in_maps, core_ids=list(range(NCORES)), trace=trace
    )
    out = np.concatenate(
        [res.results[c]["out_loc"] for c in range(NCORES)], axis=0
    )
    return out.reshape(B, D, 1).astype(np.float32), res


def kernel(**inputs):
    out, _ = _run(inputs, trace=False)
    return out


# revision 3
# speedup vs baseline: 1.0842x; 1.0485x over previous
"""Trainium2 Bass kernel for nn_MultiHeadAttention (B=8192, D=1024, 16 heads
used only via the softmax scale 1/8).

Strategy (8 NeuronCores, AllGather of locally-projected K^T only):
  - Rows (batch axis) are sharded: core c owns rows [c*1024, (c+1)*1024).
  - Each core computes Q^T and K^T for ONLY its local 1024 rows (fp16
    weights/activations, fp32 PSUM), then AllGathers K^T (fp16, two halves
    so the first half's attention can start while the second is on the
    wire). Collectives run on TOPSP/SDMA, overlapped with Wq/Wv transposes
    and the Q^T projection.
  - V is NEVER materialized: since V = x @ Wv.T + bv,
        attn @ V = (attn @ x) @ Wv.T + (rowsum attn) * bv
    so the j-contraction runs against the raw input x (full x is a kernel
    input on every core - no gather needed), and the Wv projection is
    applied once to the [1024, 1024] attn@x result in the epilogue.
  - Attention in transposed-energy ("E^T") layout, no probability
    transpose needed:
        E^T[j, i] = sum_o K^T[o, j] * Q^T[o, i]      (fp16 x fp16)
        P^T = exp(E^T * 0.125)        bf16 (range!); |E|<40 pre-exp safe
        AX[i, d] = sum_j P^T[j, i] * x[j, d]         (bf16 x bf16)
        s[i] = sum_j P^T[j, i]        (matmul against a ones vector,
                                       issued FIRST in each group so the
                                       next ldweights hides under the two
                                       512-row matmuls)
        out = (AX / s) @ Wv.T + bv                   (fp16 x fp16)
  - Phase-2 stream pools are hoisted to the outer scope so their SBUF is
    never re-used from phase-1 pools: the K/x prefetch DMAs start during
    phase 1 instead of waiting for its drain.
"""

import sys

sys.path.insert(0, "/opt/trn_rl_repo")

import numpy as np

import concourse.bass as bass  # noqa: F401
import concourse.tile as tile
from concourse import bacc, mybir
from concourse.bass_utils import run_bass_kernel_spmd
from concourse.masks import make_identity

B = 8192
D = 1024
P = 128
NCORES = 8
R = B // NCORES  # 1024 rows per core
JBLK = 256  # j-block (keys) streamed per iteration
NJB = B // JBLK  # 32
JJN = JBLK // P  # 2 j sub-chunks per block
IMOV = 512  # moving-dim chunk for the i axis (PE max)
DO = D // P  # 8 feature chunks of 128
IC = R // P  # 8 row chunks of 128 per core
NH = 4  # K^T allgather quarters
F32 = mybir.dt.float32
FP16 = mybir.dt.float16
BF16 = mybir.dt.bfloat16
AF = mybir.ActivationFunctionType
ALU = mybir.AluOpType
SCALE = 0.125  # 1/sqrt(head_dim=64)
RG = [list(range(NCORES))]


def build_program():
    nc = bacc.Bacc(
        "TRN2", target_bir_lowering=False, debug=False, num_devices=NCORES
    )
    x = nc.dram_tensor("x", [B, D], F32, kind="ExternalInput").ap()
    x_loc = nc.dram_tensor("x_loc", [R, D], F32, kind="ExternalInput").ap()
    w_q = nc.dram_tensor("Wq", [D, D], F32, kind="ExternalInput").ap()
    w_k = nc.dram_tensor("Wk", [D, D], F32, kind="ExternalInput").ap()
    w_v = nc.dram_tensor("Wv", [D, D], F32, kind="ExternalInput").ap()
    b_q = nc.dram_tensor("bq", [D], F32, kind="ExternalInput").ap()
    b_k = nc.dram_tensor("bk", [D], F32, kind="ExternalInput").ap()
    b_v = nc.dram_tensor("bv", [D], F32, kind="ExternalInput").ap()
    out_loc = nc.dram_tensor("out_loc", [R, D], F32, kind="ExternalOutput").ap()

    with tile.TileContext(nc) as tc:
        _body(nc, tc, x, x_loc, w_q, w_k, w_v, b_q, b_k, b_v, out_loc)
    nc.compile()
    return nc


def _body(nc, tc, x, x_loc, w_q, w_k, w_v, b_q, b_k, b_v, out_loc):
    from contextlib import ExitStack

    outer = ExitStack()
    outer.__enter__()
    # ---- persistent pools (whole kernel) ----
    const_pool = outer.enter_context(tc.tile_pool(name="const", bufs=1))
    identity = const_pool.tile([P, P], F32)
    make_identity(nc, identity)
    ones_tmp = const_pool.tile([P, 2], F32)
    nc.vector.memset(ones_tmp, 1.0)
    ones_bf = const_pool.tile([P, 2], BF16)
    nc.vector.tensor_copy(out=ones_bf, in_=ones_tmp)
    bq_sb = const_pool.tile([P, DO], F32)
    nc.sync.dma_start(bq_sb, b_q.rearrange("(oo p) -> p oo", p=P))
    bk_sb = const_pool.tile([P, DO], F32)
    nc.sync.dma_start(bk_sb, b_k.rearrange("(oo p) -> p oo", p=P))
    ones_row = const_pool.tile([1, P], F32)
    nc.vector.memset(ones_row, 1.0)
    # broadcast bv across all 128 partitions with a K=1 matmul:
    # load bv into partition 0 of bv_bc, then out[p, o] = 1 * bv[o]
    bv_bc = const_pool.tile([P, D], F32)
    nc.sync.dma_start(bv_bc[0:1, :], b_v[None, :])
    with tc.tile_pool(name="bv_psum", bufs=2, space="PSUM") as bvp:
        for oh in range(2):
            pt = bvp.tile([P, 512], F32, tag="bvp")
            nc.tensor.matmul(
                pt,
                ones_row,
                bv_bc[0:1, oh * 512 : (oh + 1) * 512],
                start=True,
                stop=True,
            )
            nc.vector.tensor_copy(out=bv_bc[:, oh * 512 : (oh + 1) * 512], in_=pt)

    qt_pool = outer.enter_context(tc.tile_pool(name="qt", bufs=1))
    qt = qt_pool.tile([P, DO, R], FP16)  # Q^T: [o_in, o_out, i]  (2 MB)
    wvt_pool = outer.enter_context(tc.tile_pool(name="wvt", bufs=1))
    wvt = wvt_pool.tile([P, DO, D], FP16)  # Wv^T, used only in the epilogue

    sums_pool = outer.enter_context(tc.tile_pool(name="sums", bufs=1))
    sums_acc = sums_pool.tile([P, 2 * IC], F32)  # per-row exp-sums (even cols)
    rsum = sums_pool.tile([P, 2 * IC], F32)

    # phase-2 stream pools, hoisted so their SBUF never aliases phase-1
    # pools (prefetch DMAs may start while phase 1 is still draining)
    oa_pool = outer.enter_context(tc.tile_pool(name="oacc", bufs=1))
    outacc = oa_pool.tile([P, IC, D], F32)  # AX accumulator (4 MB)
    kt_pool = outer.enter_context(tc.tile_pool(name="ktb", bufs=2))
    xr_pool = outer.enter_context(tc.tile_pool(name="xraw", bufs=2))
    xb_pool = outer.enter_context(tc.tile_pool(name="xb", bufs=2))
    pt_pool = outer.enter_context(tc.tile_pool(name="ptb", bufs=1))

    # DRAM: local K^T shard (two halves) + gathered full copies.
    dram = outer.enter_context(tc.tile_pool(name="dram", bufs=1, space="DRAM"))
    kt_part = [dram.tile([DO, P, JBLK], FP16, name=f"kt_part{h}") for h in range(NH)]
    kt_full = [
        dram.tile([NCORES, DO, P, JBLK], FP16, name=f"kt_full{h}") for h in range(NH)
    ]

    # =========================================================
    # Phase 1: weight transposes, local K^T / Q^T, AllGathers
    # =========================================================
    with ExitStack() as p1:
        wt_pool = p1.enter_context(tc.tile_pool(name="wt", bufs=1))
        xt_pool = p1.enter_context(tc.tile_pool(name="xt", bufs=1))
        xt = xt_pool.tile([P, DO, R], FP16)  # x_loc^T (2 MB)

        row_pool = p1.enter_context(tc.tile_pool(name="rows", bufs=2))
        st_pool = p1.enter_context(tc.tile_pool(name="stage", bufs=2))
        tp_psum = p1.enter_context(tc.tile_pool(name="tp_ps", bufs=2, space="PSUM"))
        mm_psum = p1.enter_context(tc.tile_pool(name="mm_ps", bufs=2, space="PSUM"))

        def transpose_weight(wt_sb, w_dram):
            for oo in range(DO):
                wrow = row_pool.tile([P, D], F32, tag="row")
                nc.sync.dma_start(wrow, w_dram[oo * P : (oo + 1) * P, :])
                for dd in range(DO):
                    tp = tp_psum.tile([P, P], F32, tag="tp")
                    nc.tensor.transpose(
                        tp, wrow[:, dd * P : (dd + 1) * P], identity
                    )
                    nc.vector.tensor_copy(
                        out=wt_sb[:, dd, oo * P : (oo + 1) * P], in_=tp
                    )

        # -- Wk first: it gates the collectives --
        wkt = wt_pool.tile([P, DO, D], FP16, tag="wt", name="wkt")
        transpose_weight(wkt, w_k)

        # -- x_loc^T for the local shard --
        for jj in range(R // P):
            xrow = row_pool.tile([P, D], F32, tag="row")
            nc.sync.dma_start(xrow, x_loc[jj * P : (jj + 1) * P, :])
            for dd in range(DO):
                tp = tp_psum.tile([P, P], F32, tag="tp")
                nc.tensor.transpose(tp, xrow[:, dd * P : (dd + 1) * P], identity)
                nc.vector.tensor_copy(out=xt[:, dd, jj * P : (jj + 1) * P], in_=tp)

        # -- local K^T shard halves: [o, j_loc], AllGather each asap --
        for h in range(NH):
            for oo in range(DO):
                pk = mm_psum.tile([P, JBLK], F32, tag="mmk")
                for dd in range(DO):
                    nc.tensor.matmul(
                        pk,
                        (wkt[:, dd, oo * P : (oo + 1) * P]),
                        (xt[:, dd, h * JBLK : (h + 1) * JBLK]),
                        start=(dd == 0),
                        stop=(dd == DO - 1),
                    )
                kst = st_pool.tile([P, JBLK], FP16, tag="kst")
                nc.scalar.activation(
                    kst, pk, AF.Identity, bias=bk_sb[:, oo : oo + 1]
                )
                nc.sync.dma_start(kt_part[h][oo, :, :], kst)
            nc.gpsimd.collective_compute(
                "AllGather",
                mybir.AluOpType.bypass,
                replica_groups=RG,
                ins=[kt_part[h][:, :, :]],
                outs=[kt_full[h][:, :, :, :]],
            )

        # -- Q^T for the local shard (overlaps the in-flight collectives) --
        wqt = wt_pool.tile([P, DO, D], FP16, tag="wt", name="wqt")
        transpose_weight(wqt, w_q)
        for ih in range(R // IMOV):
            for oo in range(DO):
                pq = mm_psum.tile([P, IMOV], F32, tag="mm")
                for dd in range(DO):
                    nc.tensor.matmul(
                        pq,
                        (wqt[:, dd, oo * P : (oo + 1) * P]),
                        (xt[:, dd, ih * IMOV : (ih + 1) * IMOV]),
                        start=(dd == 0),
                        stop=(dd == DO - 1),
                    )
                nc.scalar.activation(
                    qt[:, oo, ih * IMOV : (ih + 1) * IMOV],
                    pq,
                    AF.Identity,
                    bias=bq_sb[:, oo : oo + 1],
                )

        # -- Wv^T (only needed by the epilogue; do it last) --
        transpose_weight(wvt, w_v)

    # =========================================================
    # Phase 2: streamed attention in E^T layout, j against raw x
    # =========================================================
    with ExitStack() as p2:
        p2m = ExitStack()
        p2m.__enter__()
        e_psum = p2m.enter_context(tc.tile_pool(name="e_ps", bufs=4, space="PSUM"))
        o_psum = p2m.enter_context(tc.tile_pool(name="o_ps", bufs=3, space="PSUM"))
        s_psum = p2m.enter_context(tc.tile_pool(name="s_ps", bufs=1, space="PSUM"))

        for it in range(NJB):
            # quarter-major order: all of AG quarter 0 first, then 1, ...
            src_q, src_c = it // NCORES, it % NCORES
            row0 = src_c * R + src_q * JBLK
            ktb = kt_pool.tile([P, DO, JBLK], FP16, tag="ktb")
            for oo in range(DO):
                nc.sync.dma_start(
                    ktb[:, oo, :], kt_full[src_q][src_c, oo, :, :]
                )
            xraw = xr_pool.tile([P, JJN, D], F32, tag="xraw")
            nc.sync.dma_start(
                xraw,
                x[row0 : row0 + JBLK, :].rearrange("(jj p) o -> p jj o", p=P),
            )
            xb = xb_pool.tile([P, JJN, D], BF16, tag="xb")
            nc.gpsimd.tensor_copy(out=xb, in_=xraw)  # Pool engine: cast
            # unnormalized probabilities P^T for this j-block: [j, i]
            ptb = pt_pool.tile([P, JJN, R], BF16, tag="ptb")
            for jj in range(JJN):
                pe_h = [
                    e_psum.tile([P, IMOV], F32, tag="pe", name="pe")
                    for _ in range(R // IMOV)
                ]
                for oo in range(DO):
                    for ih in range(R // IMOV):
                        nc.tensor.matmul(
                            pe_h[ih],
                            (ktb[:, oo, jj * P : (jj + 1) * P]),
                            (qt[:, oo, ih * IMOV : (ih + 1) * IMOV]),
                            start=(oo == 0),
                            stop=(oo == DO - 1),
                        )
                for ih in range(R // IMOV):
                    nc.scalar.activation(
                        ptb[:, jj, ih * IMOV : (ih + 1) * IMOV],
                        pe_h[ih],
                        AF.Exp,
                        scale=SCALE,
                    )
            # AX += P^T.T @ x_blk. The 2-row exp-sums matmul runs FIRST in
            # each stationary group so the NEXT group's ldweights hides
            # under this group's two 512-row matmuls.
            ps = s_psum.tile([P, 2 * IC], F32, tag="ps")
            for ic in range(IC):
                po_h = [
                    o_psum.tile([P, 512], F32, tag="po", name="po")
                    for _ in range(2)
                ]
                for jj in range(JJN):
                    nc.tensor.matmul(
                        ps[:, 2 * ic : 2 * ic + 2],
                        (ptb[:, jj, ic * P : (ic + 1) * P]),
                        (ones_bf),
                        start=(ic == 0 and jj == 0),
                        stop=(ic == IC - 1 and jj == JJN - 1),
                    )
                    for oh in range(2):
                        nc.tensor.matmul(
                            po_h[oh],
                            (ptb[:, jj, ic * P : (ic + 1) * P]),
                            (xb[:, jj, oh * 512 : (oh + 1) * 512]),
                            start=(jj == 0),
                            stop=(jj == JJN - 1),
                        )
                for oh in range(2):
                    dst = outacc[:, ic, oh * 512 : (oh + 1) * 512]
                    if it == 0:
                        nc.vector.tensor_copy(out=dst, in_=po_h[oh])
                    else:
                        nc.vector.tensor_tensor(dst, po_h[oh], dst, ALU.add)
            if it == 0:
                nc.vector.tensor_copy(out=sums_acc, in_=ps)
            else:
                nc.vector.tensor_tensor(sums_acc, ps, sums_acc, ALU.add)

        p2m.close()  # release main-loop PSUM pools for the epilogue's
        # ---- epilogue: normalize, transpose AX, @ Wv^T, add bv ----
        nc.vector.reciprocal(rsum, sums_acc)
        axt_pool = p2.enter_context(tc.tile_pool(name="axt", bufs=1))
        axt = axt_pool.tile([P, DO, R], FP16)  # (AX/s)^T: [d_in, dd, i] (2 MB)
        fin_pool = p2.enter_context(tc.tile_pool(name="fin", bufs=2))
        tp2_psum = p2.enter_context(tc.tile_pool(name="tp2", bufs=2, space="PSUM"))
        f_psum = p2.enter_context(tc.tile_pool(name="f_ps", bufs=4, space="PSUM"))
        for ic in range(IC):
            nc.vector.tensor_scalar_mul(
                outacc[:, ic, :], outacc[:, ic, :], rsum[:, 2 * ic : 2 * ic + 1]
            )
            for dd in range(DO):
                tp = tp2_psum.tile([P, P], F32, tag="tp2")
                nc.tensor.transpose(
                    tp, outacc[:, ic, dd * P : (dd + 1) * P], identity
                )
                nc.vector.tensor_copy(
                    out=axt[:, dd, ic * P : (ic + 1) * P], in_=tp
                )
        for ic in range(IC):
            ofin = fin_pool.tile([P, D], F32, tag="ofin")
            for oh in range(2):
                pf = f_psum.tile([P, 512], F32, tag="f")
                for dd in range(DO):
                    nc.tensor.matmul(
                        pf,
                        (axt[:, dd, ic * P : (ic + 1) * P]),
                        (wvt[:, dd, oh * 512 : (oh + 1) * 512]),
                        start=(dd == 0),
                        stop=(dd == DO - 1),
                    )
                nc.vector.tensor_tensor(
                    ofin[:, oh * 512 : (oh + 1) * 512],
                    pf,
                    bv_bc[:, oh * 512 : (oh + 1) * 512],
                    ALU.add,
                )
            nc.sync.dma_start(out_loc[ic * P : (ic + 1) * P, :], ofin)

    outer.close()


_NC_CACHE = None


def _get_program():
    global _NC_CACHE
    if _NC_CACHE is None:
        _NC_CACHE = build_program()
    return _NC_CACHE


def _run(inputs, trace=False):
    nc = _get_program()
    x = np.ascontiguousarray(np.asarray(inputs["x"], dtype=np.float32))
    common = {
        k: np.ascontiguousarray(np.asarray(inputs[k], dtype=np.float32))
        for k in ("Wq", "Wk", "Wv", "bq", "bk", "bv")
    }
    in_maps = [
        {"x": x, "x_loc": np.ascontiguousarray(x[c * R : (c + 1) * R]), **common}
        for c in range(NCORES)
    ]
    res = run_bass_kernel_spmd(
        nc, in_maps, core_ids=list(range(NCORES)), trace=trace
    )
    out = np.concatenate(
        [res.results[c]["out_loc"] for c in range(NCORES)], axis=0
    )
    return out.reshape(B, D, 1).astype(np.float32), res


def kernel(**inputs):
    out, _ = _run(inputs, trace=False)
    return out


# revision 4
# speedup vs baseline: 1.0856x; 1.0013x over previous
"""Trainium2 Bass kernel for nn_MultiHeadAttention (B=8192, D=1024, 16 heads
used only via the softmax scale 1/8).

Strategy (8 NeuronCores, AllGather of locally-projected K^T only):
  - Rows (batch axis) are sharded: core c owns rows [c*1024, (c+1)*1024).
  - Each core computes Q^T and K^T for ONLY its local 1024 rows (fp16
    weights/activations, fp32 PSUM), then AllGathers K^T (fp16, two halves
    so the first half's attention can start while the second is on the
    wire). Collectives run on TOPSP/SDMA, overlapped with Wq/Wv transposes
    and the Q^T projection.
  - V is NEVER materialized: since V = x @ Wv.T + bv,
        attn @ V = (attn @ x) @ Wv.T + (rowsum attn) * bv
    so the j-contraction runs against the raw input x (full x is a kernel
    input on every core - no gather needed), and the Wv projection is
    applied once to the [1024, 1024] attn@x result in the epilogue.
  - Attention in transposed-energy ("E^T") layout, no probability
    transpose needed:
        E^T[j, i] = sum_o K^T[o, j] * Q^T[o, i]      (fp16 x fp16)
        P^T = exp(E^T * 0.125)        bf16 (range!); |E|<40 pre-exp safe
        AX[i, d] = sum_j P^T[j, i] * x[j, d]         (bf16 x bf16)
        s[i] = sum_j P^T[j, i]        (matmul against a ones vector,
                                       issued FIRST in each group so the
                                       next ldweights hides under the two
                                       512-row matmuls)
        out = (AX / s) @ Wv.T + bv                   (fp16 x fp16)
  - Phase-2 stream pools are hoisted to the outer scope so their SBUF is
    never re-used from phase-1 pools: the K/x prefetch DMAs start during
    phase 1 instead of waiting for its drain.
"""

import sys

sys.path.insert(0, "/opt/trn_rl_repo")

import numpy as np

import concourse.bass as bass  # noqa: F401
import concourse.tile as tile
from concourse import bacc, mybir
from concourse.bass_utils import run_bass_kernel_spmd
from concourse.masks import make_identity

B = 8192
D = 1024
P = 128
NCORES = 8
R = B // NCORES  # 1024 rows per core
JBLK = 256  # j-block (keys) streamed per iteration
NJB = B // JBLK  # 32
JJN = JBLK // P  # 2 j sub-chunks per block
IMOV = 512  # moving-dim chunk for the i axis (PE max)
DO = D // P  # 8 feature chunks of 128
IC = R // P  # 8 row chunks of 128 per core
NH = 4  # K^T allgather quarters
F32 = mybir.dt.float32
FP16 = mybir.dt.float16
BF16 = mybir.dt.bfloat16
AF = mybir.ActivationFunctionType
ALU = mybir.AluOpType
SCALE = 0.125  # 1/sqrt(head_dim=64)
RG = [list(range(NCORES))]


def build_program():
    nc = bacc.Bacc(
        "TRN2", target_bir_lowering=False, debug=False, num_devices=NCORES
    )
    x = nc.dram_tensor("x", [B, D], F32, kind="ExternalInput").ap()
    x_loc = nc.dram_tensor("x_loc", [R, D], F32, kind="ExternalInput").ap()
    w_q = nc.dram_tensor("Wq", [D, D], F32, kind="ExternalInput").ap()
    w_k = nc.dram_tensor("Wk", [D, D], F32, kind="ExternalInput").ap()
    w_v = nc.dram_tensor("Wv", [D, D], F32, kind="ExternalInput").ap()
    b_q = nc.dram_tensor("bq", [D], F32, kind="ExternalInput").ap()
    b_k = nc.dram_tensor("bk", [D], F32, kind="ExternalInput").ap()
    b_v = nc.dram_tensor("bv", [D], F32, kind="ExternalInput").ap()
    out_loc = nc.dram_tensor("out_loc", [R, D], F32, kind="ExternalOutput").ap()

    with tile.TileContext(nc) as tc:
        _body(nc, tc, x, x_loc, w_q, w_k, w_v, b_q, b_k, b_v, out_loc)
    nc.compile()
    return nc


def _body(nc, tc, x, x_loc, w_q, w_k, w_v, b_q, b_k, b_v, out_loc):
    from contextlib import ExitStack

    outer = ExitStack()
    outer.__enter__()
    # ---- persistent pools (whole kernel) ----
    const_pool = outer.enter_context(tc.tile_pool(name="const", bufs=1))
    identity = const_pool.tile([P, P], F32)
    make_identity(nc, identity)
    ones_tmp = const_pool.tile([P, 2], F32)
    nc.vector.memset(ones_tmp, 1.0)
    ones_bf = const_pool.tile([P, 2], BF16)
    nc.vector.tensor_copy(out=ones_bf, in_=ones_tmp)
    bq_sb = const_pool.tile([P, DO], F32)
    nc.sync.dma_start(bq_sb, b_q.rearrange("(oo p) -> p oo", p=P))
    bk_sb = const_pool.tile([P, DO], F32)
    nc.sync.dma_start(bk_sb, b_k.rearrange("(oo p) -> p oo", p=P))
    ones_row = const_pool.tile([1, P], F32)
    nc.vector.memset(ones_row, 1.0)
    # broadcast bv across all 128 partitions with a K=1 matmul:
    # load bv into partition 0 of bv_bc, then out[p, o] = 1 * bv[o]
    bv_bc = const_pool.tile([P, D], F32)
    nc.sync.dma_start(bv_bc[0:1, :], b_v[None, :])
    with tc.tile_pool(name="bv_psum", bufs=2, space="PSUM") as bvp:
        for oh in range(2):
            pt = bvp.tile([P, 512], F32, tag="bvp")
            nc.tensor.matmul(
                pt,
                ones_row,
                bv_bc[0:1, oh * 512 : (oh + 1) * 512],
                start=True,
                stop=True,
            )
            nc.vector.tensor_copy(out=bv_bc[:, oh * 512 : (oh + 1) * 512], in_=pt)

    qt_pool = outer.enter_context(tc.tile_pool(name="qt", bufs=1))
    qt = qt_pool.tile([P, DO, R], FP16)  # Q^T: [o_in, o_out, i]  (2 MB)
    wvt_pool = outer.enter_context(tc.tile_pool(name="wvt", bufs=1))
    wvt = wvt_pool.tile([P, DO, D], FP16)  # Wv^T, used only in the epilogue

    sums_pool = outer.enter_context(tc.tile_pool(name="sums", bufs=1))
    sums_acc = sums_pool.tile([P, 2 * IC], F32)  # per-row exp-sums (even cols)
    rsum = sums_pool.tile([P, 2 * IC], F32)

    # phase-2 stream pools, hoisted so their SBUF never aliases phase-1
    # pools (prefetch DMAs may start while phase 1 is still draining)
    oa_pool = outer.enter_context(tc.tile_pool(name="oacc", bufs=1))
    outacc = oa_pool.tile([P, IC, D], F32)  # AX accumulator (4 MB)
    kt_pool = outer.enter_context(tc.tile_pool(name="ktb", bufs=2))
    xr_pool = outer.enter_context(tc.tile_pool(name="xraw", bufs=2))
    xb_pool = outer.enter_context(tc.tile_pool(name="xb", bufs=2))
    pt_pool = outer.enter_context(tc.tile_pool(name="ptb", bufs=1))

    # DRAM: local K^T shard (two halves) + gathered full copies.
    dram = outer.enter_context(tc.tile_pool(name="dram", bufs=1, space="DRAM"))
    kt_part = [dram.tile([DO, P, JBLK], FP16, name=f"kt_part{h}") for h in range(NH)]
    kt_full = [
        dram.tile(
            [NCORES, DO, P, JBLK], FP16, name=f"kt_full{h}", addr_space="Shared"
        )
        for h in range(NH)
    ]

    # =========================================================
    # Phase 1: weight transposes, local K^T / Q^T, AllGathers
    # =========================================================
    with ExitStack() as p1:
        wt_pool = p1.enter_context(tc.tile_pool(name="wt", bufs=1))
        xt_pool = p1.enter_context(tc.tile_pool(name="xt", bufs=1))
        xt = xt_pool.tile([P, DO, R], FP16)  # x_loc^T (2 MB)

        row_pool = p1.enter_context(tc.tile_pool(name="rows", bufs=4))
        st_pool = p1.enter_context(tc.tile_pool(name="stage", bufs=2))
        tp_psum = p1.enter_context(tc.tile_pool(name="tp_ps", bufs=2, space="PSUM"))
        mm_psum = p1.enter_context(tc.tile_pool(name="mm_ps", bufs=2, space="PSUM"))

        def transpose_weight(wt_sb, w_dram):
            for oo in range(DO):
                wrow = row_pool.tile([P, D], F32, tag="row")
                nc.sync.dma_start(wrow, w_dram[oo * P : (oo + 1) * P, :])
                for dd in range(DO):
                    tp = tp_psum.tile([P, P], F32, tag="tp")
                    nc.tensor.transpose(
                        tp, wrow[:, dd * P : (dd + 1) * P], identity
                    )
                    nc.vector.tensor_copy(
                        out=wt_sb[:, dd, oo * P : (oo + 1) * P], in_=tp
                    )

        # -- Wk first: it gates the collectives --
        wkt = wt_pool.tile([P, DO, D], FP16, tag="wt", name="wkt")
        transpose_weight(wkt, w_k)

        # -- x_loc^T for the local shard --
        for jj in range(R // P):
            xrow = row_pool.tile([P, D], F32, tag="row")
            nc.sync.dma_start(xrow, x_loc[jj * P : (jj + 1) * P, :])
            for dd in range(DO):
                tp = tp_psum.tile([P, P], F32, tag="tp")
                nc.tensor.transpose(tp, xrow[:, dd * P : (dd + 1) * P], identity)
                nc.vector.tensor_copy(out=xt[:, dd, jj * P : (jj + 1) * P], in_=tp)

        # -- local K^T shard halves: [o, j_loc], AllGather each asap --
        for h in range(NH):
            for oo in range(DO):
                pk = mm_psum.tile([P, JBLK], F32, tag="mmk")
                for dd in range(DO):
                    nc.tensor.matmul(
                        pk,
                        (wkt[:, dd, oo * P : (oo + 1) * P]),
                        (xt[:, dd, h * JBLK : (h + 1) * JBLK]),
                        start=(dd == 0),
                        stop=(dd == DO - 1),
                    )
                kst = st_pool.tile([P, JBLK], FP16, tag="kst")
                nc.scalar.activation(
                    kst, pk, AF.Identity, bias=bk_sb[:, oo : oo + 1]
                )
                nc.sync.dma_start(kt_part[h][oo, :, :], kst)
            nc.gpsimd.collective_compute(
                "AllGather",
                mybir.AluOpType.bypass,
                replica_groups=RG,
                ins=[kt_part[h][:, :, :]],
                outs=[kt_full[h][:, :, :, :]],
            )

        # -- Q^T for the local shard (overlaps the in-flight collectives) --
        wqt = wt_pool.tile([P, DO, D], FP16, tag="wt", name="wqt")
        transpose_weight(wqt, w_q)
        for ih in range(R // IMOV):
            for oo in range(DO):
                pq = mm_psum.tile([P, IMOV], F32, tag="mm")
                for dd in range(DO):
                    nc.tensor.matmul(
                        pq,
                        (wqt[:, dd, oo * P : (oo + 1) * P]),
                        (xt[:, dd, ih * IMOV : (ih + 1) * IMOV]),
                        start=(dd == 0),
                        stop=(dd == DO - 1),
                    )
                nc.scalar.activation(
                    qt[:, oo, ih * IMOV : (ih + 1) * IMOV],
                    pq,
                    AF.Identity,
                    bias=bq_sb[:, oo : oo + 1],
                )

        # -- Wv^T (only needed by the epilogue; do it last) --
        transpose_weight(wvt, w_v)

    # =========================================================
    # Phase 2: streamed attention in E^T layout, j against raw x
    # =========================================================
    with ExitStack() as p2:
        p2m = ExitStack()
        p2m.__enter__()
        e_psum = p2m.enter_context(tc.tile_pool(name="e_ps", bufs=4, space="PSUM"))
        o_psum = p2m.enter_context(tc.tile_pool(name="o_ps", bufs=3, space="PSUM"))
        s_psum = p2m.enter_context(tc.tile_pool(name="s_ps", bufs=1, space="PSUM"))

        for it in range(NJB):
            # quarter-major order: all of AG quarter 0 first, then 1, ...
            src_q, src_c = it // NCORES, it % NCORES
            row0 = src_c * R + src_q * JBLK
            ktb = kt_pool.tile([P, DO, JBLK], FP16, tag="ktb")
            for oo in range(DO):
                nc.sync.dma_start(
                    ktb[:, oo, :], kt_full[src_q][src_c, oo, :, :]
                )
            xraw = xr_pool.tile([P, JJN, D], F32, tag="xraw")
            nc.sync.dma_start(
                xraw,
                x[row0 : row0 + JBLK, :].rearrange("(jj p) o -> p jj o", p=P),
            )
            xb = xb_pool.tile([P, JJN, D], BF16, tag="xb")
            nc.gpsimd.tensor_copy(out=xb, in_=xraw)  # Pool engine: cast
            # unnormalized probabilities P^T for this j-block: [j, i]
            ptb = pt_pool.tile([P, JJN, R], BF16, tag="ptb")
            for jj in range(JJN):
                pe_h = [
                    e_psum.tile([P, IMOV], F32, tag="pe", name="pe")
                    for _ in range(R // IMOV)
                ]
                for oo in range(DO):
                    for ih in range(R // IMOV):
                        nc.tensor.matmul(
                            pe_h[ih],
                            (ktb[:, oo, jj * P : (jj + 1) * P]),
                            (qt[:, oo, ih * IMOV : (ih + 1) * IMOV]),
                            start=(oo == 0),
                            stop=(oo == DO - 1),
                        )
                for ih in range(R // IMOV):
                    nc.scalar.activation(
                        ptb[:, jj, ih * IMOV : (ih + 1) * IMOV],
                        pe_h[ih],
                        AF.Exp,
                        scale=SCALE,
                    )
            # AX += P^T.T @ x_blk. The 2-row exp-sums matmul runs FIRST in
            # each stationary group so the NEXT group's ldweights hides
            # under this group's two 512-row matmuls.
            ps = s_psum.tile([P, 2 * IC], F32, tag="ps")
            for ic in range(IC):
                po_h = [
                    o_psum.tile([P, 512], F32, tag="po", name="po")
                    for _ in range(2)
                ]
                for jj in range(JJN):
                    nc.tensor.matmul(
                        ps[:, 2 * ic : 2 * ic + 2],
                        (ptb[:, jj, ic * P : (ic + 1) * P]),
                        (ones_bf),
                        start=(ic == 0 and jj == 0),
                        stop=(ic == IC - 1 and jj == JJN - 1),
                    )
                    for oh in range(2):
                        nc.tensor.matmul(
                            po_h[oh],
                            (ptb[:, jj, ic * P : (ic + 1) * P]),
                            (xb[:, jj, oh * 512 : (oh + 1) * 512]),
                            start=(jj == 0),
                            stop=(jj == JJN - 1),
                        )
                for oh in range(2):
                    dst = outacc[:, ic, oh * 512 : (oh + 1) * 512]
                    if it == 0:
                        nc.vector.tensor_copy(out=dst, in_=po_h[oh])
                    else:
                        nc.vector.tensor_tensor(dst, po_h[oh], dst, ALU.add)
            if it == 0:
                nc.vector.tensor_copy(out=sums_acc, in_=ps)
            else:
                nc.vector.tensor_tensor(sums_acc, ps, sums_acc, ALU.add)

        p2m.close()  # release main-loop PSUM pools for the epilogue's
        # ---- epilogue: normalize, transpose AX, @ Wv^T, add bv ----
        nc.vector.reciprocal(rsum, sums_acc)
        axt_pool = p2.enter_context(tc.tile_pool(name="axt", bufs=1))
        axt = axt_pool.tile([P, DO, R], FP16)  # (AX/s)^T: [d_in, dd, i] (2 MB)
        fin_pool = p2.enter_context(tc.tile_pool(name="fin", bufs=2))
        tp2_psum = p2.enter_context(tc.tile_pool(name="tp2", bufs=2, space="PSUM"))
        f_psum = p2.enter_context(tc.tile_pool(name="f_ps", bufs=4, space="PSUM"))
        for ic in range(IC):
            nc.vector.tensor_scalar_mul(
                outacc[:, ic, :], outacc[:, ic, :], rsum[:, 2 * ic : 2 * ic + 1]
            )
            for dd in range(DO):
                tp = tp2_psum.tile([P, P], F32, tag="tp2")
                nc.tensor.transpose(
                    tp, outacc[:, ic, dd * P : (dd + 1) * P], identity
                )
                nc.vector.tensor_copy(
                    out=axt[:, dd, ic * P : (ic + 1) * P], in_=tp
                )
        for ic in range(IC):
            ofin = fin_pool.tile([P, D], F32, tag="ofin")
            for oh in range(2):
                pf = f_psum.tile([P, 512], F32, tag="f")
                for dd in range(DO):
                    nc.tensor.matmul(
                        pf,
                        (axt[:, dd, ic * P : (ic + 1) * P]),
                        (wvt[:, dd, oh * 512 : (oh + 1) * 512]),
                        start=(dd == 0),
                        stop=(dd == DO - 1),
                    )
                nc.vector.tensor_tensor(
                    ofin[:, oh * 512 : (oh + 1) * 512],
                    pf,
                    bv_bc[:, oh * 512 : (oh + 1) * 512],
                    ALU.add,
                )
            nc.sync.dma_start(out_loc[ic * P : (ic + 1) * P, :], ofin)

    outer.close()


_NC_CACHE = None


def _get_program():
    global _NC_CACHE
    if _NC_CACHE is None:
        _NC_CACHE = build_program()
    return _NC_CACHE


def _run(inputs, trace=False):
    nc = _get_program()
    x = np.ascontiguousarray(np.asarray(inputs["x"], dtype=np.float32))
    common = {
        k: np.ascontiguousarray(np.asarray(inputs[k], dtype=np.float32))
        for k in ("Wq", "Wk", "Wv", "bq", "bk", "bv")
    }
    in_maps = [
        {"x": x, "x_loc": np.ascontiguousarray(x[c * R : (c + 1) * R]), **common}
        for c in range(NCORES)
    ]
    res = run_bass_kernel_spmd(
        nc, in_maps, core_ids=list(range(NCORES)), trace=trace
    )
    out = np.concatenate(
        [res.results[c]["out_loc"] for c in range(NCORES)], axis=0
    )
    return out.reshape(B, D, 1).astype(np.float32), res


def kernel(**inputs):
    out, _ = _run(inputs, trace=False)
    return out


# revision 5
# speedup vs baseline: 1.1319x; 1.0426x over previous
"""Trainium2 Bass kernel for nn_MultiHeadAttention (B=8192, D=1024, 16 heads
used only via the softmax scale 1/8).

Strategy (8 NeuronCores, AllGather of locally-projected K^T only):
  - Rows (batch axis) are sharded: core c owns rows [c*1024, (c+1)*1024).
  - Each core computes Q^T and K^T for ONLY its local 1024 rows (fp16
    weights/activations, fp32 PSUM), then AllGathers K^T (fp16, two halves
    so the first half's attention can start while the second is on the
    wire). Collectives run on TOPSP/SDMA, overlapped with Wq/Wv transposes
    and the Q^T projection.
  - V is NEVER materialized: since V = x @ Wv.T + bv,
        attn @ V = (attn @ x) @ Wv.T + (rowsum attn) * bv
    so the j-contraction runs against the raw input x (full x is a kernel
    input on every core - no gather needed), and the Wv projection is
    applied once to the [1024, 1024] attn@x result in the epilogue.
  - Attention in transposed-energy ("E^T") layout, no probability
    transpose needed:
        E^T[j, i] = sum_o K^T[o, j] * Q^T[o, i]      (fp16 x fp16)
        P^T = exp(E^T * 0.125)        bf16 (range!); |E|<40 pre-exp safe
        AX[i, d] = sum_j P^T[j, i] * x[j, d]         (bf16 x bf16)
        s[i] = sum_j P^T[j, i]        (matmul against a ones vector,
                                       issued FIRST in each group so the
                                       next ldweights hides under the two
                                       512-row matmuls)
        out = (AX / s) @ Wv.T + bv                   (fp16 x fp16)
  - Phase-2 stream pools are hoisted to the outer scope so their SBUF is
    never re-used from phase-1 pools: the K/x prefetch DMAs start during
    phase 1 instead of waiting for its drain.
"""

import sys

sys.path.insert(0, "/opt/trn_rl_repo")

import numpy as np

import concourse.bass as bass  # noqa: F401
import concourse.tile as tile
from concourse import bacc, mybir
from concourse.bass_utils import run_bass_kernel_spmd
from concourse.masks import make_identity

B = 8192
D = 1024
P = 128
NCORES = 8
R = B // NCORES  # 1024 rows per core
JBLK = 256  # j-block (keys) streamed per iteration
NJB = B // JBLK  # 32
JJN = JBLK // P  # 2 j sub-chunks per block
IMOV = 512  # moving-dim chunk for the i axis (PE max)
DO = D // P  # 8 feature chunks of 128
IC = R // P  # 8 row chunks of 128 per core
NH = 4  # K^T allgather quarters
F32 = mybir.dt.float32
FP16 = mybir.dt.float16
BF16 = mybir.dt.bfloat16
AF = mybir.ActivationFunctionType
ALU = mybir.AluOpType
SCALE = 0.125  # 1/sqrt(head_dim=64)
RG = [list(range(NCORES))]


def build_program():
    nc = bacc.Bacc(
        "TRN2", target_bir_lowering=False, debug=False, num_devices=NCORES
    )
    x = nc.dram_tensor("x", [B, D], F32, kind="ExternalInput").ap()
    x_loc = nc.dram_tensor("x_loc", [R, D], F32, kind="ExternalInput").ap()
    w_q = nc.dram_tensor("Wq", [D, D], F32, kind="ExternalInput").ap()
    w_k = nc.dram_tensor("Wk", [D, D], F32, kind="ExternalInput").ap()
    w_v = nc.dram_tensor("Wv", [D, D], F32, kind="ExternalInput").ap()
    b_q = nc.dram_tensor("bq", [D], F32, kind="ExternalInput").ap()
    b_k = nc.dram_tensor("bk", [D], F32, kind="ExternalInput").ap()
    b_v = nc.dram_tensor("bv", [D], F32, kind="ExternalInput").ap()
    out_loc = nc.dram_tensor("out_loc", [R, D], F32, kind="ExternalOutput").ap()

    with tile.TileContext(nc) as tc:
        _body(nc, tc, x, x_loc, w_q, w_k, w_v, b_q, b_k, b_v, out_loc)
    nc.compile()
    return nc


def _body(nc, tc, x, x_loc, w_q, w_k, w_v, b_q, b_k, b_v, out_loc):
    from contextlib import ExitStack

    outer = ExitStack()
    outer.__enter__()
    # ---- persistent pools (whole kernel) ----
    const_pool = outer.enter_context(tc.tile_pool(name="const", bufs=1))
    identity = const_pool.tile([P, P], F32)
    make_identity(nc, identity)
    ones_tmp = const_pool.tile([P, 2], F32)
    nc.vector.memset(ones_tmp, 1.0)
    ones_bf = const_pool.tile([P, 2], BF16)
    nc.vector.tensor_copy(out=ones_bf, in_=ones_tmp)
    bq_sb = const_pool.tile([P, DO], F32)
    nc.sync.dma_start(bq_sb, b_q.rearrange("(oo p) -> p oo", p=P))
    bk_sb = const_pool.tile([P, DO], F32)
    nc.sync.dma_start(bk_sb, b_k.rearrange("(oo p) -> p oo", p=P))
    ones_row = const_pool.tile([1, P], F32)
    nc.vector.memset(ones_row, 1.0)
    # broadcast bv across all 128 partitions with a K=1 matmul:
    # load bv into partition 0 of bv_bc, then out[p, o] = 1 * bv[o]
    bv_bc = const_pool.tile([P, D], F32)
    nc.sync.dma_start(bv_bc[0:1, :], b_v[None, :])
    with tc.tile_pool(name="bv_psum", bufs=2, space="PSUM") as bvp:
        for oh in range(2):
            pt = bvp.tile([P, 512], F32, tag="bvp")
            nc.tensor.matmul(
                pt,
                ones_row,
                bv_bc[0:1, oh * 512 : (oh + 1) * 512],
                start=True,
                stop=True,
            )
            nc.vector.tensor_copy(out=bv_bc[:, oh * 512 : (oh + 1) * 512], in_=pt)

    qt_pool = outer.enter_context(tc.tile_pool(name="qt", bufs=1))
    qt = qt_pool.tile([P, DO, R], FP16)  # Q^T: [o_in, o_out, i]  (2 MB)
    wvt_pool = outer.enter_context(tc.tile_pool(name="wvt", bufs=1))
    wvt = wvt_pool.tile([P, DO, D], FP16)  # Wv^T, used only in the epilogue

    sums_pool = outer.enter_context(tc.tile_pool(name="sums", bufs=1))
    sums_acc = sums_pool.tile([P, 2 * IC], F32)  # per-row exp-sums (even cols)
    rsum = sums_pool.tile([P, 2 * IC], F32)

    # phase-2 stream pools, hoisted so their SBUF never aliases phase-1
    # pools (prefetch DMAs may start while phase 1 is still draining)
    oa_pool = outer.enter_context(tc.tile_pool(name="oacc", bufs=1))
    outacc = oa_pool.tile([P, IC, D], F32)  # AX accumulator (4 MB)
    kt_pool = outer.enter_context(tc.tile_pool(name="ktb", bufs=2))
    xr_pool = outer.enter_context(tc.tile_pool(name="xraw", bufs=2))
    xb_pool = outer.enter_context(tc.tile_pool(name="xb", bufs=2))
    pt_pool = outer.enter_context(tc.tile_pool(name="ptb", bufs=1))

    # DRAM: local K^T shard (two halves) + gathered full copies.
    dram = outer.enter_context(tc.tile_pool(name="dram", bufs=1, space="DRAM"))
    kt_part = [dram.tile([DO, P, JBLK], FP16, name=f"kt_part{h}") for h in range(NH)]
    kt_full = [
        dram.tile(
            [NCORES, DO, P, JBLK], FP16, name=f"kt_full{h}", addr_space="Shared"
        )
        for h in range(NH)
    ]

    # =========================================================
    # Phase 1: weight transposes, local K^T / Q^T, AllGathers
    # =========================================================
    with ExitStack() as p1:
        wt_pool = p1.enter_context(tc.tile_pool(name="wt", bufs=1))
        xt_pool = p1.enter_context(tc.tile_pool(name="xt", bufs=1))
        xt = xt_pool.tile([P, DO, R], FP16)  # x_loc^T (2 MB)

        row_pool = p1.enter_context(tc.tile_pool(name="rows", bufs=6))
        st_pool = p1.enter_context(tc.tile_pool(name="stage", bufs=2))
        tp_psum = p1.enter_context(tc.tile_pool(name="tp_ps", bufs=2, space="PSUM"))
        mm_psum = p1.enter_context(tc.tile_pool(name="mm_ps", bufs=2, space="PSUM"))

        def transpose_weight(wt_sb, w_dram):
            for oo in range(DO):
                wrow = row_pool.tile([P, D], F32, tag="row")
                nc.sync.dma_start(wrow, w_dram[oo * P : (oo + 1) * P, :])
                for dd in range(DO):
                    tp = tp_psum.tile([P, P], F32, tag="tp")
                    nc.tensor.transpose(
                        tp, wrow[:, dd * P : (dd + 1) * P], identity
                    )
                    nc.vector.tensor_copy(
                        out=wt_sb[:, dd, oo * P : (oo + 1) * P], in_=tp
                    )

        # -- Wk first: it gates the collectives --
        wkt = wt_pool.tile([P, DO, D], FP16, tag="wt", name="wkt")
        transpose_weight(wkt, w_k)

        # -- x_loc^T for the local shard --
        for jj in range(R // P):
            xrow = row_pool.tile([P, D], F32, tag="row")
            nc.sync.dma_start(xrow, x_loc[jj * P : (jj + 1) * P, :])
            for dd in range(DO):
                tp = tp_psum.tile([P, P], F32, tag="tp")
                nc.tensor.transpose(tp, xrow[:, dd * P : (dd + 1) * P], identity)
                nc.vector.tensor_copy(out=xt[:, dd, jj * P : (jj + 1) * P], in_=tp)

        # -- local K^T shard: 512-row matmuls (fewer PE drains), with the
        #    activation/DMA writes split per 256-row AG quarter --
        for hh in range(NH // 2):
            for oo in range(DO):
                pk = mm_psum.tile([P, 2 * JBLK], F32, tag="mmk")
                for dd in range(DO):
                    nc.tensor.matmul(
                        pk,
                        (wkt[:, dd, oo * P : (oo + 1) * P]),
                        (xt[:, dd, hh * 2 * JBLK : (hh + 1) * 2 * JBLK]),
                        start=(dd == 0),
                        stop=(dd == DO - 1),
                    )
                kst = st_pool.tile([P, 2 * JBLK], FP16, tag="kst")
                nc.scalar.activation(
                    kst, pk, AF.Identity, bias=bk_sb[:, oo : oo + 1]
                )
                for sub in range(2):
                    nc.sync.dma_start(
                        kt_part[2 * hh + sub][oo, :, :],
                        kst[:, sub * JBLK : (sub + 1) * JBLK],
                    )
            for sub in range(2):
                h = 2 * hh + sub
                nc.gpsimd.collective_compute(
                    "AllGather",
                    mybir.AluOpType.bypass,
                    replica_groups=RG,
                    ins=[kt_part[h][:, :, :]],
                    outs=[kt_full[h][:, :, :, :]],
                )

        # -- Q^T for the local shard (overlaps the in-flight collectives) --
        wqt = wt_pool.tile([P, DO, D], FP16, tag="wt", name="wqt")
        transpose_weight(wqt, w_q)
        for ih in range(R // IMOV):
            for oo in range(DO):
                pq = mm_psum.tile([P, IMOV], F32, tag="mm")
                for dd in range(DO):
                    nc.tensor.matmul(
                        pq,
                        (wqt[:, dd, oo * P : (oo + 1) * P]),
                        (xt[:, dd, ih * IMOV : (ih + 1) * IMOV]),
                        start=(dd == 0),
                        stop=(dd == DO - 1),
                    )
                nc.scalar.activation(
                    qt[:, oo, ih * IMOV : (ih + 1) * IMOV],
                    pq,
                    AF.Identity,
                    bias=bq_sb[:, oo : oo + 1],
                )

        # -- Wv^T (only needed by the epilogue; do it last) --
        transpose_weight(wvt, w_v)

    # =========================================================
    # Phase 2: streamed attention in E^T layout, j against raw x
    # =========================================================
    with ExitStack() as p2:
        p2m = ExitStack()
        p2m.__enter__()
        e_psum = p2m.enter_context(tc.tile_pool(name="e_ps", bufs=4, space="PSUM"))
        o_psum = p2m.enter_context(tc.tile_pool(name="o_ps", bufs=3, space="PSUM"))
        s_psum = p2m.enter_context(tc.tile_pool(name="s_ps", bufs=1, space="PSUM"))

        for it in range(NJB):
            # quarter-major order: all of AG quarter 0 first, then 1, ...
            src_q, src_c = it // NCORES, it % NCORES
            row0 = src_c * R + src_q * JBLK
            ktb = kt_pool.tile([P, DO, JBLK], FP16, tag="ktb")
            for oo in range(DO):
                nc.sync.dma_start(
                    ktb[:, oo, :], kt_full[src_q][src_c, oo, :, :]
                )
            xraw = xr_pool.tile([P, JJN, D], F32, tag="xraw")
            nc.sync.dma_start(
                xraw,
                x[row0 : row0 + JBLK, :].rearrange("(jj p) o -> p jj o", p=P),
            )
            xb = xb_pool.tile([P, JJN, D], BF16, tag="xb")
            nc.gpsimd.tensor_copy(out=xb, in_=xraw)  # Pool engine: cast
            # unnormalized probabilities P^T for this j-block: [j, i]
            ptb = pt_pool.tile([P, JJN, R], BF16, tag="ptb")
            for jj in range(JJN):
                pe_h = [
                    e_psum.tile([P, IMOV], F32, tag="pe", name="pe")
                    for _ in range(R // IMOV)
                ]
                for oo in range(DO):
                    for ih in range(R // IMOV):
                        nc.tensor.matmul(
                            pe_h[ih],
                            (ktb[:, oo, jj * P : (jj + 1) * P]),
                            (qt[:, oo, ih * IMOV : (ih + 1) * IMOV]),
                            start=(oo == 0),
                            stop=(oo == DO - 1),
                        )
                for ih in range(R // IMOV):
                    nc.scalar.activation(
                        ptb[:, jj, ih * IMOV : (ih + 1) * IMOV],
                        pe_h[ih],
                        AF.Exp,
                        scale=SCALE,
                    )
            # AX += P^T.T @ x_blk. The 2-row exp-sums matmul runs FIRST in
            # each stationary group so the NEXT group's ldweights hides
            # under this group's two 512-row matmuls.
            ps = s_psum.tile([P, 2 * IC], F32, tag="ps")
            for ic in range(IC):
                po_h = [
                    o_psum.tile([P, 512], F32, tag="po", name="po")
                    for _ in range(2)
                ]
                for jj in range(JJN):
                    nc.tensor.matmul(
                        ps[:, 2 * ic : 2 * ic + 2],
                        (ptb[:, jj, ic * P : (ic + 1) * P]),
                        (ones_bf),
                        start=(ic == 0 and jj == 0),
                        stop=(ic == IC - 1 and jj == JJN - 1),
                    )
                    for oh in range(2):
                        nc.tensor.matmul(
                            po_h[oh],
                            (ptb[:, jj, ic * P : (ic + 1) * P]),
                            (xb[:, jj, oh * 512 : (oh + 1) * 512]),
                            start=(jj == 0),
                            stop=(jj == JJN - 1),
                        )
                for oh in range(2):
                    dst = outacc[:, ic, oh * 512 : (oh + 1) * 512]
                    if it == 0:
                        nc.vector.tensor_copy(out=dst, in_=po_h[oh])
                    else:
                        nc.vector.tensor_tensor(dst, po_h[oh], dst, ALU.add)
            if it == 0:
                nc.vector.tensor_copy(out=sums_acc, in_=ps)
            else:
                nc.vector.tensor_tensor(sums_acc, ps, sums_acc, ALU.add)

        p2m.close()  # release main-loop PSUM pools for the epilogue's
        # ---- epilogue: normalize, transpose AX, @ Wv^T, add bv ----
        nc.vector.reciprocal(rsum, sums_acc)
        axt_pool = p2.enter_context(tc.tile_pool(name="axt", bufs=1))
        axt = axt_pool.tile([P, DO, R], FP16)  # (AX/s)^T: [d_in, dd, i] (2 MB)
        fin_pool = p2.enter_context(tc.tile_pool(name="fin", bufs=2))
        tp2_psum = p2.enter_context(tc.tile_pool(name="tp2", bufs=2, space="PSUM"))
        f_psum = p2.enter_context(tc.tile_pool(name="f_ps", bufs=4, space="PSUM"))
        for ic in range(IC):
            nc.vector.tensor_scalar_mul(
                outacc[:, ic, :], outacc[:, ic, :], rsum[:, 2 * ic : 2 * ic + 1]
            )
            for dd in range(DO):
                tp = tp2_psum.tile([P, P], F32, tag="tp2")
                nc.tensor.transpose(
                    tp, outacc[:, ic, dd * P : (dd + 1) * P], identity
                )
                nc.vector.tensor_copy(
                    out=axt[:, dd, ic * P : (ic + 1) * P], in_=tp
                )
        for ic in range(IC):
            ofin = fin_pool.tile([P, D], F32, tag="ofin")
            for oh in range(2):
                pf = f_psum.tile([P, 512], F32, tag="f")
                for dd in range(DO):
                    nc.tensor.matmul(
                        pf,
                        (axt[:, dd, ic * P : (ic + 1) * P]),
                        (wvt[:, dd, oh * 512 : (oh + 1) * 512]),
                        start=(dd == 0),
                        stop=(dd == DO - 1),
                    )
                nc.vector.tensor_tensor(
                    ofin[:, oh * 512 : (oh + 1) * 512],
                    pf,
                    bv_bc[:, oh * 512 : (oh + 1) * 512],
                    ALU.add,
                )
            nc.sync.dma_start(out_loc[ic * P : (ic + 1) * P, :], ofin)

    outer.close()


_NC_CACHE = None


def _get_program():
    global _NC_CACHE
    if _NC_CACHE is None:
        _NC_CACHE = build_program()
    return _NC_CACHE


def _run(inputs, trace=False):
    nc = _get_program()
    x = np.ascontiguousarray(np.asarray(inputs["x"], dtype=np.float32))
    common = {
        k: np.ascontiguousarray(np.asarray(inputs[k], dtype=np.float32))
        for k in ("Wq", "Wk", "Wv", "bq", "bk", "bv")
    }
    in_maps = [
        {"x": x, "x_loc": np.ascontiguousarray(x[c * R : (c + 1) * R]), **common}
        for c in range(NCORES)
    ]
    res = run_bass_kernel_spmd(
        nc, in_maps, core_ids=list(range(NCORES)), trace=trace
    )
    out = np.concatenate(
        [res.results[c]["out_loc"] for c in range(NCORES)], axis=0
    )
    return out.reshape(B, D, 1).astype(np.float32), res


def kernel(**inputs):
    out, _ = _run(inputs, trace=False)
    return out


# revision 6
# speedup vs baseline: 1.1345x; 1.0023x over previous
"""Trainium2 Bass kernel for nn_MultiHeadAttention (B=8192, D=1024, 16 heads
used only via the softmax scale 1/8).

Strategy (8 NeuronCores, AllGather of locally-projected K^T only):
  - Rows (batch axis) are sharded: core c owns rows [c*1024, (c+1)*1024).
  - Each core computes Q^T and K^T for ONLY its local 1024 rows (fp16
    weights/activations, fp32 PSUM), then AllGathers K^T (fp16, two halves
    so the first half's attention can start while the second is on the
    wire). Collectives run on TOPSP/SDMA, overlapped with Wq/Wv transposes
    and the Q^T projection.
  - V is NEVER materialized: since V = x @ Wv.T + bv,
        attn @ V = (attn @ x) @ Wv.T + (rowsum attn) * bv
    so the j-contraction runs against the raw input x (full x is a kernel
    input on every core - no gather needed), and the Wv projection is
    applied once to the [1024, 1024] attn@x result in the epilogue.
  - Attention in transposed-energy ("E^T") layout, no probability
    transpose needed:
        E^T[j, i] = sum_o K^T[o, j] * Q^T[o, i]      (fp16 x fp16)
        P^T = exp(E^T * 0.125)        bf16 (range!); |E|<40 pre-exp safe
        AX[i, d] = sum_j P^T[j, i] * x[j, d]         (bf16 x bf16)
        s[i] = sum_j P^T[j, i]        (matmul against a ones vector,
                                       issued FIRST in each group so the
                                       next ldweights hides under the two
                                       512-row matmuls)
        out = (AX / s) @ Wv.T + bv                   (fp16 x fp16)
  - Phase-2 stream pools are hoisted to the outer scope so their SBUF is
    never re-used from phase-1 pools: the K/x prefetch DMAs start during
    phase 1 instead of waiting for its drain.
"""

import sys

sys.path.insert(0, "/opt/trn_rl_repo")

import numpy as np

import concourse.bass as bass  # noqa: F401
import concourse.tile as tile
from concourse import bacc, mybir
from concourse.bass_utils import run_bass_kernel_spmd
from concourse.masks import make_identity

B = 8192
D = 1024
P = 128
NCORES = 8
R = B // NCORES  # 1024 rows per core
JBLK = 256  # j-block (keys) streamed per iteration
NJB = B // JBLK  # 32
JJN = JBLK // P  # 2 j sub-chunks per block
IMOV = 512  # moving-dim chunk for the i axis (PE max)
DO = D // P  # 8 feature chunks of 128
IC = R // P  # 8 row chunks of 128 per core
NH = 4  # K^T allgather quarters
F32 = mybir.dt.float32
FP16 = mybir.dt.float16
BF16 = mybir.dt.bfloat16
AF = mybir.ActivationFunctionType
ALU = mybir.AluOpType
SCALE = 0.125  # 1/sqrt(head_dim=64)
RG = [list(range(NCORES))]


def build_program():
    nc = bacc.Bacc(
        "TRN2", target_bir_lowering=False, debug=False, num_devices=NCORES
    )
    x = nc.dram_tensor("x", [B, D], F32, kind="ExternalInput").ap()
    x_loc = nc.dram_tensor("x_loc", [R, D], F32, kind="ExternalInput").ap()
    w_q = nc.dram_tensor("Wq", [D, D], F32, kind="ExternalInput").ap()
    w_k = nc.dram_tensor("Wk", [D, D], F32, kind="ExternalInput").ap()
    w_v = nc.dram_tensor("Wv", [D, D], F32, kind="ExternalInput").ap()
    b_q = nc.dram_tensor("bq", [D], F32, kind="ExternalInput").ap()
    b_k = nc.dram_tensor("bk", [D], F32, kind="ExternalInput").ap()
    b_v = nc.dram_tensor("bv", [D], F32, kind="ExternalInput").ap()
    out_loc = nc.dram_tensor("out_loc", [R, D], F32, kind="ExternalOutput").ap()

    with tile.TileContext(nc) as tc:
        _body(nc, tc, x, x_loc, w_q, w_k, w_v, b_q, b_k, b_v, out_loc)
    nc.compile()
    return nc


def _body(nc, tc, x, x_loc, w_q, w_k, w_v, b_q, b_k, b_v, out_loc):
    from contextlib import ExitStack

    outer = ExitStack()
    outer.__enter__()
    # ---- persistent pools (whole kernel) ----
    const_pool = outer.enter_context(tc.tile_pool(name="const", bufs=1))
    identity = const_pool.tile([P, P], F32)
    make_identity(nc, identity)
    identity16 = const_pool.tile([P, P], FP16)
    nc.vector.tensor_copy(out=identity16, in_=identity)
    ones_tmp = const_pool.tile([P, 2], F32)
    nc.vector.memset(ones_tmp, 1.0)
    ones_bf = const_pool.tile([P, 2], BF16)
    nc.vector.tensor_copy(out=ones_bf, in_=ones_tmp)
    bq_sb = const_pool.tile([P, DO], F32)
    nc.sync.dma_start(bq_sb, b_q.rearrange("(oo p) -> p oo", p=P))
    bk_sb = const_pool.tile([P, DO], F32)
    nc.sync.dma_start(bk_sb, b_k.rearrange("(oo p) -> p oo", p=P))
    ones_row = const_pool.tile([1, P], F32)
    nc.vector.memset(ones_row, 1.0)
    # broadcast bv across all 128 partitions with a K=1 matmul:
    # load bv into partition 0 of bv_bc, then out[p, o] = 1 * bv[o]
    bv_bc = const_pool.tile([P, D], F32)
    nc.sync.dma_start(bv_bc[0:1, :], b_v[None, :])
    with tc.tile_pool(name="bv_psum", bufs=2, space="PSUM") as bvp:
        for oh in range(2):
            pt = bvp.tile([P, 512], F32, tag="bvp")
            nc.tensor.matmul(
                pt,
                ones_row,
                bv_bc[0:1, oh * 512 : (oh + 1) * 512],
                start=True,
                stop=True,
            )
            nc.vector.tensor_copy(out=bv_bc[:, oh * 512 : (oh + 1) * 512], in_=pt)

    qt_pool = outer.enter_context(tc.tile_pool(name="qt", bufs=1))
    qt = qt_pool.tile([P, DO, R], FP16)  # Q^T: [o_in, o_out, i]  (2 MB)
    wvt_pool = outer.enter_context(tc.tile_pool(name="wvt", bufs=1))
    wvt = wvt_pool.tile([P, DO, D], FP16)  # Wv^T, used only in the epilogue

    sums_pool = outer.enter_context(tc.tile_pool(name="sums", bufs=1))
    sums_acc = sums_pool.tile([P, 2 * IC], F32)  # per-row exp-sums (even cols)
    rsum = sums_pool.tile([P, 2 * IC], F32)

    # phase-2 stream pools, hoisted so their SBUF never aliases phase-1
    # pools (prefetch DMAs may start while phase 1 is still draining)
    oa_pool = outer.enter_context(tc.tile_pool(name="oacc", bufs=1))
    outacc = oa_pool.tile([P, IC, D], F32)  # AX accumulator (4 MB)
    kt_pool = outer.enter_context(tc.tile_pool(name="ktb", bufs=2))
    xr_pool = outer.enter_context(tc.tile_pool(name="xraw", bufs=2))
    xb_pool = outer.enter_context(tc.tile_pool(name="xb", bufs=2))
    pt_pool = outer.enter_context(tc.tile_pool(name="ptb", bufs=1))

    # DRAM: local K^T shard (two halves) + gathered full copies.
    dram = outer.enter_context(tc.tile_pool(name="dram", bufs=1, space="DRAM"))
    kt_part = [dram.tile([DO, P, JBLK], FP16, name=f"kt_part{h}") for h in range(NH)]
    kt_full = [
        dram.tile(
            [NCORES, DO, P, JBLK], FP16, name=f"kt_full{h}", addr_space="Shared"
        )
        for h in range(NH)
    ]

    # =========================================================
    # Phase 1: weight transposes, local K^T / Q^T, AllGathers
    # =========================================================
    with ExitStack() as p1:
        wt_pool = p1.enter_context(tc.tile_pool(name="wt", bufs=1))
        xt_pool = p1.enter_context(tc.tile_pool(name="xt", bufs=1))
        xt = xt_pool.tile([P, DO, R], FP16)  # x_loc^T (2 MB)

        row_pool = p1.enter_context(tc.tile_pool(name="rows", bufs=6))
        st_pool = p1.enter_context(tc.tile_pool(name="stage", bufs=2))
        tp_psum = p1.enter_context(tc.tile_pool(name="tp_ps", bufs=2, space="PSUM"))
        mm_psum = p1.enter_context(tc.tile_pool(name="mm_ps", bufs=2, space="PSUM"))

        def transpose_weight(wt_sb, w_dram):
            # cast the f32 row to fp16 once (DVE), then transpose at the
            # PE's 1-cycle/row fp16 rate with a matching fp16 identity
            for oo in range(DO):
                wrow = row_pool.tile([P, D], F32, tag="row")
                nc.sync.dma_start(wrow, w_dram[oo * P : (oo + 1) * P, :])
                wrow16 = row_pool.tile([P, D], FP16, tag="row16")
                nc.vector.tensor_copy(out=wrow16, in_=wrow)
                for dd in range(DO):
                    tp = tp_psum.tile([P, P], FP16, tag="tp16")
                    nc.tensor.transpose(
                        tp, wrow16[:, dd * P : (dd + 1) * P], identity16
                    )
                    nc.vector.tensor_copy(
                        out=wt_sb[:, dd, oo * P : (oo + 1) * P], in_=tp
                    )

        # -- Wk first: it gates the collectives --
        wkt = wt_pool.tile([P, DO, D], FP16, tag="wt", name="wkt")
        transpose_weight(wkt, w_k)

        # -- x_loc^T for the local shard --
        for jj in range(R // P):
            xrow = row_pool.tile([P, D], F32, tag="row")
            nc.sync.dma_start(xrow, x_loc[jj * P : (jj + 1) * P, :])
            xrow16 = row_pool.tile([P, D], FP16, tag="row16")
            nc.vector.tensor_copy(out=xrow16, in_=xrow)
            for dd in range(DO):
                tp = tp_psum.tile([P, P], FP16, tag="tp16")
                nc.tensor.transpose(
                    tp, xrow16[:, dd * P : (dd + 1) * P], identity16
                )
                nc.vector.tensor_copy(out=xt[:, dd, jj * P : (jj + 1) * P], in_=tp)

        # -- local K^T shard: 512-row matmuls (fewer PE drains), with the
        #    activation/DMA writes split per 256-row AG quarter --
        for hh in range(NH // 2):
            for oo in range(DO):
                pk = mm_psum.tile([P, 2 * JBLK], F32, tag="mmk")
                for dd in range(DO):
                    nc.tensor.matmul(
                        pk,
                        (wkt[:, dd, oo * P : (oo + 1) * P]),
                        (xt[:, dd, hh * 2 * JBLK : (hh + 1) * 2 * JBLK]),
                        start=(dd == 0),
                        stop=(dd == DO - 1),
                    )
                kst = st_pool.tile([P, 2 * JBLK], FP16, tag="kst")
                nc.scalar.activation(
                    kst, pk, AF.Identity, bias=bk_sb[:, oo : oo + 1]
                )
                for sub in range(2):
                    nc.sync.dma_start(
                        kt_part[2 * hh + sub][oo, :, :],
                        kst[:, sub * JBLK : (sub + 1) * JBLK],
                    )
            for sub in range(2):
                h = 2 * hh + sub
                nc.gpsimd.collective_compute(
                    "AllGather",
                    mybir.AluOpType.bypass,
                    replica_groups=RG,
                    ins=[kt_part[h][:, :, :]],
                    outs=[kt_full[h][:, :, :, :]],
                )

        # -- Q^T for the local shard (overlaps the in-flight collectives) --
        wqt = wt_pool.tile([P, DO, D], FP16, tag="wt", name="wqt")
        transpose_weight(wqt, w_q)
        for ih in range(R // IMOV):
            for oo in range(DO):
                pq = mm_psum.tile([P, IMOV], F32, tag="mm")
                for dd in range(DO):
                    nc.tensor.matmul(
                        pq,
                        (wqt[:, dd, oo * P : (oo + 1) * P]),
                        (xt[:, dd, ih * IMOV : (ih + 1) * IMOV]),
                        start=(dd == 0),
                        stop=(dd == DO - 1),
                    )
                nc.scalar.activation(
                    qt[:, oo, ih * IMOV : (ih + 1) * IMOV],
                    pq,
                    AF.Identity,
                    bias=bq_sb[:, oo : oo + 1],
                )

        # -- Wv^T (only needed by the epilogue; do it last) --
        transpose_weight(wvt, w_v)

    # =========================================================
    # Phase 2: streamed attention in E^T layout, j against raw x
    # =========================================================
    with ExitStack() as p2:
        p2m = ExitStack()
        p2m.__enter__()
        e_psum = p2m.enter_context(tc.tile_pool(name="e_ps", bufs=4, space="PSUM"))
        o_psum = p2m.enter_context(tc.tile_pool(name="o_ps", bufs=3, space="PSUM"))
        s_psum = p2m.enter_context(tc.tile_pool(name="s_ps", bufs=1, space="PSUM"))

        for it in range(NJB):
            # quarter-major order: all of AG quarter 0 first, then 1, ...
            src_q, src_c = it // NCORES, it % NCORES
            row0 = src_c * R + src_q * JBLK
            ktb = kt_pool.tile([P, DO, JBLK], FP16, tag="ktb")
            for oo in range(DO):
                nc.sync.dma_start(
                    ktb[:, oo, :], kt_full[src_q][src_c, oo, :, :]
                )
            xraw = xr_pool.tile([P, JJN, D], F32, tag="xraw")
            nc.sync.dma_start(
                xraw,
                x[row0 : row0 + JBLK, :].rearrange("(jj p) o -> p jj o", p=P),
            )
            xb = xb_pool.tile([P, JJN, D], BF16, tag="xb")
            nc.gpsimd.tensor_copy(out=xb, in_=xraw)  # Pool engine: cast
            # unnormalized probabilities P^T for this j-block: [j, i]
            ptb = pt_pool.tile([P, JJN, R], BF16, tag="ptb")
            for jj in range(JJN):
                pe_h = [
                    e_psum.tile([P, IMOV], F32, tag="pe", name="pe")
                    for _ in range(R // IMOV)
                ]
                for oo in range(DO):
                    for ih in range(R // IMOV):
                        nc.tensor.matmul(
                            pe_h[ih],
                            (ktb[:, oo, jj * P : (jj + 1) * P]),
                            (qt[:, oo, ih * IMOV : (ih + 1) * IMOV]),
                            start=(oo == 0),
                            stop=(oo == DO - 1),
                        )
                for ih in range(R // IMOV):
                    nc.scalar.activation(
                        ptb[:, jj, ih * IMOV : (ih + 1) * IMOV],
                        pe_h[ih],
                        AF.Exp,
                        scale=SCALE,
                    )
            # AX += P^T.T @ x_blk. The 2-row exp-sums matmul runs FIRST in
            # each stationary group so the NEXT group's ldweights hides
            # under this group's two 512-row matmuls.
            ps = s_psum.tile([P, 2 * IC], F32, tag="ps")
            for ic in range(IC):
                po_h = [
                    o_psum.tile([P, 512], F32, tag="po", name="po")
                    for _ in range(2)
                ]
                for jj in range(JJN):
                    nc.tensor.matmul(
                        ps[:, 2 * ic : 2 * ic + 2],
                        (ptb[:, jj, ic * P : (ic + 1) * P]),
                        (ones_bf),
                        start=(ic == 0 and jj == 0),
                        stop=(ic == IC - 1 and jj == JJN - 1),
                    )
                    for oh in range(2):
                        nc.tensor.matmul(
                            po_h[oh],
                            (ptb[:, jj, ic * P : (ic + 1) * P]),
                            (xb[:, jj, oh * 512 : (oh + 1) * 512]),
                            start=(jj == 0),
                            stop=(jj == JJN - 1),
                        )
                for oh in range(2):
                    dst = outacc[:, ic, oh * 512 : (oh + 1) * 512]
                    if it == 0:
                        nc.vector.tensor_copy(out=dst, in_=po_h[oh])
                    else:
                        nc.vector.tensor_tensor(dst, po_h[oh], dst, ALU.add)
            if it == 0:
                nc.vector.tensor_copy(out=sums_acc, in_=ps)
            else:
                nc.vector.tensor_tensor(sums_acc, ps, sums_acc, ALU.add)

        p2m.close()  # release main-loop PSUM pools for the epilogue's
        # ---- epilogue: normalize, transpose AX, @ Wv^T, add bv ----
        nc.vector.reciprocal(rsum, sums_acc)
        axt_pool = p2.enter_context(tc.tile_pool(name="axt", bufs=1))
        axt = axt_pool.tile([P, DO, R], FP16)  # (AX/s)^T: [d_in, dd, i] (2 MB)
        fin_pool = p2.enter_context(tc.tile_pool(name="fin", bufs=2))
        tp2_psum = p2.enter_context(tc.tile_pool(name="tp2", bufs=2, space="PSUM"))
        f_psum = p2.enter_context(tc.tile_pool(name="f_ps", bufs=4, space="PSUM"))
        for ic in range(IC):
            nc.vector.tensor_scalar_mul(
                outacc[:, ic, :], outacc[:, ic, :], rsum[:, 2 * ic : 2 * ic + 1]
            )
            for dd in range(DO):
                tp = tp2_psum.tile([P, P], F32, tag="tp2")
                nc.tensor.transpose(
                    tp, outacc[:, ic, dd * P : (dd + 1) * P], identity
                )
                nc.vector.tensor_copy(
                    out=axt[:, dd, ic * P : (ic + 1) * P], in_=tp
                )
        for ic in range(IC):
            ofin = fin_pool.tile([P, D], F32, tag="ofin")
            for oh in range(2):
                pf = f_psum.tile([P, 512], F32, tag="f")
                for dd in range(DO):
                    nc.tensor.matmul(
                        pf,
                        (axt[:, dd, ic * P : (ic + 1) * P]),
                        (wvt[:, dd, oh * 512 : (oh + 1) * 512]),
                        start=(dd == 0),
                        stop=(dd == DO - 1),
                    )
                nc.vector.tensor_tensor(
                    ofin[:, oh * 512 : (oh + 1) * 512],
                    pf,
                    bv_bc[:, oh * 512 : (oh + 1) * 512],
                    ALU.add,
                )
            nc.sync.dma_start(out_loc[ic * P : (ic + 1) * P, :], ofin)

    outer.close()


_NC_CACHE = None


def _get_program():
    global _NC_CACHE
    if _NC_CACHE is None:
        _NC_CACHE = build_program()
    return _NC_CACHE


def _run(inputs, trace=False):
    nc = _get_program()
    x = np.ascontiguousarray(np.asarray(inputs["x"], dtype=np.float32))
    common = {
        k: np.ascontiguousarray(np.asarray(inputs[k], dtype=np.float32))
        for k in ("Wq", "Wk", "Wv", "bq", "bk", "bv")
    }
    in_maps = [
        {"x": x, "x_loc": np.ascontiguousarray(x[c * R : (c + 1) * R]), **common}
        for c in range(NCORES)
    ]
    res = run_bass_kernel_spmd(
        nc, in_maps, core_ids=list(range(NCORES)), trace=trace
    )
    out = np.concatenate(
        [res.results[c]["out_loc"] for c in range(NCORES)], axis=0
    )
    return out.reshape(B, D, 1).astype(np.float32), res


def kernel(**inputs):
    out, _ = _run(inputs, trace=False)
    return out


# revision 7
# speedup vs baseline: 1.1368x; 1.0020x over previous
"""Trainium2 Bass kernel for nn_MultiHeadAttention (B=8192, D=1024, 16 heads
used only via the softmax scale 1/8).

Strategy (8 NeuronCores, AllGather of locally-projected K^T only):
  - Rows (batch axis) are sharded: core c owns rows [c*1024, (c+1)*1024).
  - Each core computes Q^T and K^T for ONLY its local 1024 rows (fp16
    weights/activations, fp32 PSUM), then AllGathers K^T (fp16, in four
    256-row quarters with addr_space="Shared" outputs, so the first
    quarter's attention starts while the rest are on the wire).
    Collectives run on TOPSP/SDMA, overlapped with Wq/Wv transposes and
    the Q^T projection.
  - V is NEVER materialized: since V = x @ Wv.T + bv,
        attn @ V = (attn @ x) @ Wv.T + (rowsum attn) * bv
    so the j-contraction runs against the raw input x (full x is a kernel
    input on every core - no gather needed), and the Wv projection is
    applied once to the [1024, 1024] attn@x result in the epilogue.
  - Attention in transposed-energy ("E^T") layout, no probability
    transpose needed:
        E^T[j, i] = sum_o K^T[o, j] * Q^T[o, i]      (fp16 x fp16)
        P^T = exp(E^T * 0.125)        bf16 (range!); |E|<40 pre-exp safe
        AX[i, d] = sum_j P^T[j, i] * x[j, d]         (bf16 x bf16)
        s[i] = sum_j P^T[j, i]        (matmul against a ones vector,
                                       issued FIRST in each group so the
                                       next ldweights hides under the two
                                       512-row matmuls)
        out = (AX / s) @ Wv.T + bv                   (fp16 x fp16)
  - Phase-2 stream pools are hoisted to the outer scope so their SBUF is
    never re-used from phase-1 pools: the K/x prefetch DMAs start during
    phase 1 instead of waiting for its drain.
"""

import sys

sys.path.insert(0, "/opt/trn_rl_repo")

import numpy as np

import concourse.bass as bass  # noqa: F401
import concourse.tile as tile
from concourse import bacc, mybir
from concourse.bass_utils import run_bass_kernel_spmd
from concourse.masks import make_identity

B = 8192
D = 1024
P = 128
NCORES = 8
R = B // NCORES  # 1024 rows per core
JBLK = 256  # j-block (keys) streamed per iteration
NJB = B // JBLK  # 32
JJN = JBLK // P  # 2 j sub-chunks per block
IMOV = 512  # moving-dim chunk for the i axis (PE max)
DO = D // P  # 8 feature chunks of 128
IC = R // P  # 8 row chunks of 128 per core
NH = 4  # K^T allgather quarters
F32 = mybir.dt.float32
FP16 = mybir.dt.float16
BF16 = mybir.dt.bfloat16
AF = mybir.ActivationFunctionType
ALU = mybir.AluOpType
SCALE = 0.125  # 1/sqrt(head_dim=64)
RG = [list(range(NCORES))]


def build_program():
    nc = bacc.Bacc(
        "TRN2", target_bir_lowering=False, debug=False, num_devices=NCORES
    )
    x = nc.dram_tensor("x", [B, D], F32, kind="ExternalInput").ap()
    x_loc = nc.dram_tensor("x_loc", [R, D], F32, kind="ExternalInput").ap()
    w_q = nc.dram_tensor("Wq", [D, D], F32, kind="ExternalInput").ap()
    w_k = nc.dram_tensor("Wk", [D, D], F32, kind="ExternalInput").ap()
    w_v = nc.dram_tensor("Wv", [D, D], F32, kind="ExternalInput").ap()
    b_q = nc.dram_tensor("bq", [D], F32, kind="ExternalInput").ap()
    b_k = nc.dram_tensor("bk", [D], F32, kind="ExternalInput").ap()
    b_v = nc.dram_tensor("bv", [D], F32, kind="ExternalInput").ap()
    out_loc = nc.dram_tensor("out_loc", [R, D], F32, kind="ExternalOutput").ap()

    with tile.TileContext(nc) as tc:
        _body(nc, tc, x, x_loc, w_q, w_k, w_v, b_q, b_k, b_v, out_loc)
    nc.compile()
    return nc


def _body(nc, tc, x, x_loc, w_q, w_k, w_v, b_q, b_k, b_v, out_loc):
    from contextlib import ExitStack

    outer = ExitStack()
    outer.__enter__()
    # ---- persistent pools (whole kernel) ----
    const_pool = outer.enter_context(tc.tile_pool(name="const", bufs=1))
    identity = const_pool.tile([P, P], F32)
    make_identity(nc, identity)
    identity16 = const_pool.tile([P, P], FP16)
    nc.vector.tensor_copy(out=identity16, in_=identity)
    ones_tmp = const_pool.tile([P, 2], F32)
    nc.vector.memset(ones_tmp, 1.0)
    ones_bf = const_pool.tile([P, 2], BF16)
    nc.vector.tensor_copy(out=ones_bf, in_=ones_tmp)
    bq_sb = const_pool.tile([P, DO], F32)
    nc.sync.dma_start(bq_sb, b_q.rearrange("(oo p) -> p oo", p=P))
    bk_sb = const_pool.tile([P, DO], F32)
    nc.sync.dma_start(bk_sb, b_k.rearrange("(oo p) -> p oo", p=P))
    ones_row = const_pool.tile([1, P], F32)
    nc.vector.memset(ones_row, 1.0)
    # broadcast bv across all 128 partitions with a K=1 matmul:
    # load bv into partition 0 of bv_bc, then out[p, o] = 1 * bv[o]
    bv_bc = const_pool.tile([P, D], F32)
    nc.sync.dma_start(bv_bc[0:1, :], b_v[None, :])
    with tc.tile_pool(name="bv_psum", bufs=2, space="PSUM") as bvp:
        for oh in range(2):
            pt = bvp.tile([P, 512], F32, tag="bvp")
            nc.tensor.matmul(
                pt,
                ones_row,
                bv_bc[0:1, oh * 512 : (oh + 1) * 512],
                start=True,
                stop=True,
            )
            nc.vector.tensor_copy(out=bv_bc[:, oh * 512 : (oh + 1) * 512], in_=pt)

    qt_pool = outer.enter_context(tc.tile_pool(name="qt", bufs=1))
    qt = qt_pool.tile([P, DO, R], FP16)  # Q^T: [o_in, o_out, i]  (2 MB)
    wvt_pool = outer.enter_context(tc.tile_pool(name="wvt", bufs=1))
    wvt = wvt_pool.tile([P, DO, D], FP16)  # Wv^T, used only in the epilogue

    sums_pool = outer.enter_context(tc.tile_pool(name="sums", bufs=1))
    sums_acc = sums_pool.tile([P, 2 * IC], F32)  # per-row exp-sums (even cols)
    rsum = sums_pool.tile([P, 2 * IC], F32)

    # phase-2 stream pools, hoisted so their SBUF never aliases phase-1
    # pools (prefetch DMAs may start while phase 1 is still draining)
    oa_pool = outer.enter_context(tc.tile_pool(name="oacc", bufs=1))
    outacc = oa_pool.tile([P, IC, D], F32)  # AX accumulator (4 MB)
    kt_pool = outer.enter_context(tc.tile_pool(name="ktb", bufs=2))
    xr_pool = outer.enter_context(tc.tile_pool(name="xraw", bufs=2))
    xb_pool = outer.enter_context(tc.tile_pool(name="xb", bufs=2))
    pt_pool = outer.enter_context(tc.tile_pool(name="ptb", bufs=1))

    # DRAM: local K^T shard (two halves) + gathered full copies.
    dram = outer.enter_context(tc.tile_pool(name="dram", bufs=1, space="DRAM"))
    kt_part = [dram.tile([DO, P, JBLK], FP16, name=f"kt_part{h}") for h in range(NH)]
    kt_full = [
        dram.tile(
            [NCORES, DO, P, JBLK], FP16, name=f"kt_full{h}", addr_space="Shared"
        )
        for h in range(NH)
    ]

    # =========================================================
    # Phase 1: weight transposes, local K^T / Q^T, AllGathers
    # =========================================================
    with ExitStack() as p1:
        wt_pool = p1.enter_context(tc.tile_pool(name="wt", bufs=1))
        xt_pool = p1.enter_context(tc.tile_pool(name="xt", bufs=1))
        xt = xt_pool.tile([P, DO, R], FP16)  # x_loc^T (2 MB)

        row_pool = p1.enter_context(tc.tile_pool(name="rows", bufs=6))
        st_pool = p1.enter_context(tc.tile_pool(name="stage", bufs=2))
        tp_psum = p1.enter_context(tc.tile_pool(name="tp_ps", bufs=2, space="PSUM"))
        mm_psum = p1.enter_context(tc.tile_pool(name="mm_ps", bufs=2, space="PSUM"))

        def transpose_weight(wt_sb, w_dram):
            # cast the f32 row to fp16 once (DVE), then transpose at the
            # PE's 1-cycle/row fp16 rate with a matching fp16 identity
            for oo in range(DO):
                wrow = row_pool.tile([P, D], F32, tag="row")
                nc.sync.dma_start(wrow, w_dram[oo * P : (oo + 1) * P, :])
                wrow16 = row_pool.tile([P, D], FP16, tag="row16")
                nc.vector.tensor_copy(out=wrow16, in_=wrow)
                for dd in range(DO):
                    tp = tp_psum.tile([P, P], FP16, tag="tp16")
                    nc.tensor.transpose(
                        tp, wrow16[:, dd * P : (dd + 1) * P], identity16
                    )
                    nc.vector.tensor_copy(
                        out=wt_sb[:, dd, oo * P : (oo + 1) * P], in_=tp
                    )

        # -- Wk first: it gates the collectives --
        wkt = wt_pool.tile([P, DO, D], FP16, tag="wt", name="wkt")
        transpose_weight(wkt, w_k)

        # -- x_loc^T for the local shard --
        for jj in range(R // P):
            xrow = row_pool.tile([P, D], F32, tag="row")
            nc.sync.dma_start(xrow, x_loc[jj * P : (jj + 1) * P, :])
            xrow16 = row_pool.tile([P, D], FP16, tag="row16")
            nc.vector.tensor_copy(out=xrow16, in_=xrow)
            for dd in range(DO):
                tp = tp_psum.tile([P, P], FP16, tag="tp16")
                nc.tensor.transpose(
                    tp, xrow16[:, dd * P : (dd + 1) * P], identity16
                )
                nc.vector.tensor_copy(out=xt[:, dd, jj * P : (jj + 1) * P], in_=tp)

        # -- local K^T shard: 512-row matmuls (fewer PE drains), with the
        #    activation/DMA writes split per 256-row AG quarter --
        for hh in range(NH // 2):
            for oo in range(DO):
                pk = mm_psum.tile([P, 2 * JBLK], F32, tag="mmk")
                for dd in range(DO):
                    nc.tensor.matmul(
                        pk,
                        (wkt[:, dd, oo * P : (oo + 1) * P]),
                        (xt[:, dd, hh * 2 * JBLK : (hh + 1) * 2 * JBLK]),
                        start=(dd == 0),
                        stop=(dd == DO - 1),
                    )
                kst = st_pool.tile([P, 2 * JBLK], FP16, tag="kst")
                nc.scalar.activation(
                    kst, pk, AF.Identity, bias=bk_sb[:, oo : oo + 1]
                )
                for sub in range(2):
                    nc.sync.dma_start(
                        kt_part[2 * hh + sub][oo, :, :],
                        kst[:, sub * JBLK : (sub + 1) * JBLK],
                    )
            for sub in range(2):
                h = 2 * hh + sub
                nc.gpsimd.collective_compute(
                    "AllGather",
                    mybir.AluOpType.bypass,
                    replica_groups=RG,
                    ins=[kt_part[h][:, :, :]],
                    outs=[kt_full[h][:, :, :, :]],
                )

        # -- Q^T for the local shard (overlaps the in-flight collectives) --
        wqt = wt_pool.tile([P, DO, D], FP16, tag="wt", name="wqt")
        transpose_weight(wqt, w_q)
        for ih in range(R // IMOV):
            for oo in range(DO):
                pq = mm_psum.tile([P, IMOV], F32, tag="mm")
                for dd in range(DO):
                    nc.tensor.matmul(
                        pq,
                        (wqt[:, dd, oo * P : (oo + 1) * P]),
                        (xt[:, dd, ih * IMOV : (ih + 1) * IMOV]),
                        start=(dd == 0),
                        stop=(dd == DO - 1),
                    )
                nc.scalar.activation(
                    qt[:, oo, ih * IMOV : (ih + 1) * IMOV],
                    pq,
                    AF.Identity,
                    bias=bq_sb[:, oo : oo + 1],
                )

        # -- Wv^T (only needed by the epilogue; do it last) --
        transpose_weight(wvt, w_v)

    # =========================================================
    # Phase 2: streamed attention in E^T layout, j against raw x
    # =========================================================
    with ExitStack() as p2:
        p2m = ExitStack()
        p2m.__enter__()
        e_psum = p2m.enter_context(tc.tile_pool(name="e_ps", bufs=4, space="PSUM"))
        o_psum = p2m.enter_context(tc.tile_pool(name="o_ps", bufs=3, space="PSUM"))
        s_psum = p2m.enter_context(tc.tile_pool(name="s_ps", bufs=1, space="PSUM"))

        for it in range(NJB):
            # quarter-major order: all of AG quarter 0 first, then 1, ...
            src_q, src_c = it // NCORES, it % NCORES
            row0 = src_c * R + src_q * JBLK
            ktb = kt_pool.tile([P, DO, JBLK], FP16, tag="ktb")
            for oo in range(DO):
                nc.sync.dma_start(
                    ktb[:, oo, :], kt_full[src_q][src_c, oo, :, :]
                )
            xraw = xr_pool.tile([P, JJN, D], F32, tag="xraw")
            nc.sync.dma_start(
                xraw,
                x[row0 : row0 + JBLK, :].rearrange("(jj p) o -> p jj o", p=P),
            )
            xb = xb_pool.tile([P, JJN, D], BF16, tag="xb")
            nc.gpsimd.tensor_copy(out=xb, in_=xraw)  # Pool engine: cast
            # unnormalized probabilities P^T for this j-block: [j, i]
            ptb = pt_pool.tile([P, JJN, R], BF16, tag="ptb")
            for jj in range(JJN):
                pe_h = [
                    e_psum.tile([P, IMOV], F32, tag="pe", name="pe")
                    for _ in range(R // IMOV)
                ]
                for oo in range(DO):
                    for ih in range(R // IMOV):
                        nc.tensor.matmul(
                            pe_h[ih],
                            (ktb[:, oo, jj * P : (jj + 1) * P]),
                            (qt[:, oo, ih * IMOV : (ih + 1) * IMOV]),
                            start=(oo == 0),
                            stop=(oo == DO - 1),
                        )
                for ih in range(R // IMOV):
                    nc.scalar.activation(
                        ptb[:, jj, ih * IMOV : (ih + 1) * IMOV],
                        pe_h[ih],
                        AF.Exp,
                        scale=SCALE,
                    )
            # AX += P^T.T @ x_blk. The 2-row exp-sums matmul runs FIRST in
            # each stationary group so the NEXT group's ldweights hides
            # under this group's two 512-row matmuls.
            ps = s_psum.tile([P, 2 * IC], F32, tag="ps")
            for ic in range(IC):
                po_h = [
                    o_psum.tile([P, 512], F32, tag="po", name="po")
                    for _ in range(2)
                ]
                for jj in range(JJN):
                    nc.tensor.matmul(
                        ps[:, 2 * ic : 2 * ic + 2],
                        (ptb[:, jj, ic * P : (ic + 1) * P]),
                        (ones_bf),
                        start=(ic == 0 and jj == 0),
                        stop=(ic == IC - 1 and jj == JJN - 1),
                    )
                    for oh in range(2):
                        nc.tensor.matmul(
                            po_h[oh],
                            (ptb[:, jj, ic * P : (ic + 1) * P]),
                            (xb[:, jj, oh * 512 : (oh + 1) * 512]),
                            start=(jj == 0),
                            stop=(jj == JJN - 1),
                        )
                for oh in range(2):
                    dst = outacc[:, ic, oh * 512 : (oh + 1) * 512]
                    if it == 0:
                        nc.vector.tensor_copy(out=dst, in_=po_h[oh])
                    else:
                        nc.vector.tensor_tensor(dst, po_h[oh], dst, ALU.add)
            if it == 0:
                nc.vector.tensor_copy(out=sums_acc, in_=ps)
            else:
                nc.vector.tensor_tensor(sums_acc, ps, sums_acc, ALU.add)

        p2m.close()  # release main-loop PSUM pools for the epilogue's
        # ---- epilogue: normalize, transpose AX, @ Wv^T, add bv ----
        nc.vector.reciprocal(rsum, sums_acc)
        axt_pool = p2.enter_context(tc.tile_pool(name="axt", bufs=1))
        axt = axt_pool.tile([P, DO, R], FP16)  # (AX/s)^T: [d_in, dd, i] (2 MB)
        fin_pool = p2.enter_context(tc.tile_pool(name="fin", bufs=2))
        tp2_psum = p2.enter_context(tc.tile_pool(name="tp2", bufs=2, space="PSUM"))
        f_psum = p2.enter_context(tc.tile_pool(name="f_ps", bufs=4, space="PSUM"))
        for ic in range(IC):
            # one DVE op: normalize AND cast to fp16 (AX/s is O(1), safe)
            ax16 = fin_pool.tile([P, D], FP16, tag="ax16")
            nc.vector.tensor_scalar_mul(
                ax16, outacc[:, ic, :], rsum[:, 2 * ic : 2 * ic + 1]
            )
            for dd in range(DO):
                tp = tp2_psum.tile([P, P], FP16, tag="tp2")
                nc.tensor.transpose(
                    tp, ax16[:, dd * P : (dd + 1) * P], identity16
                )
                nc.vector.tensor_copy(
                    out=axt[:, dd, ic * P : (ic + 1) * P], in_=tp
                )
        for ic in range(IC):
            ofin = fin_pool.tile([P, D], F32, tag="ofin")
            for oh in range(2):
                pf = f_psum.tile([P, 512], F32, tag="f")
                for dd in range(DO):
                    nc.tensor.matmul(
                        pf,
                        (axt[:, dd, ic * P : (ic + 1) * P]),
                        (wvt[:, dd, oh * 512 : (oh + 1) * 512]),
                        start=(dd == 0),
                        stop=(dd == DO - 1),
                    )
                nc.vector.tensor_tensor(
                    ofin[:, oh * 512 : (oh + 1) * 512],
                    pf,
                    bv_bc[:, oh * 512 : (oh + 1) * 512],
                    ALU.add,
                )
            nc.sync.dma_start(out_loc[ic * P : (ic + 1) * P, :], ofin)

    outer.close()


_NC_CACHE = None


def _get_program():
    global _NC_CACHE
    if _NC_CACHE is None:
        _NC_CACHE = build_program()
    return _NC_CACHE


def _run(inputs, trace=False):
    nc = _get_program()
    x = np.ascontiguousarray(np.asarray(inputs["x"], dtype=np.float32))
    common = {
        k: np.ascontiguousarray(np.asarray(inputs[k], dtype=np.float32))
        for k in ("Wq", "Wk", "Wv", "bq", "bk", "bv")
    }
    in_maps = [
        {"x": x, "x_loc": np.ascontiguousarray(x[c * R : (c + 1) * R]), **common}
        for c in range(NCORES)
    ]
    res = run_bass_kernel_spmd(
        nc, in_maps, core_ids=list(range(NCORES)), trace=trace
    )
    out = np.concatenate(
        [res.results[c]["out_loc"] for c in range(NCORES)], axis=0
    )
    return out.reshape(B, D, 1).astype(np.float32), res


def kernel(**inputs):
    out, _ = _run(inputs, trace=False)
    return out
